# revision 11
# baseline (speedup 1.0000x reference)
# DualEdgeConv Trainium2 Bass kernel.
# Data-parallel over batch: 1 batch per NeuronCore (B=8, 8 cores).
# Per core: KNN (augmented matmul + MAX8/FIND_INDEX8/MATCH_REPLACE8 top-16),
# on-chip ap_gather edge gathers, global BN stats via AllReduce collective.
import sys

sys.path.insert(0, "/opt/trn_rl_repo")

import numpy as np

import concourse.bass as bass
import concourse.mybir as mybir
import concourse.tile as tile
from concourse import bacc
from concourse.bass_utils import run_bass_kernel_spmd
from concourse.masks import make_identity

N_CORES = 8
N = 4096          # nodes per batch
PIN = 64          # pos feature dim
EIN = 32          # ene feature dim
K = 16            # neighbors
P = 128           # partitions
NT = N // P       # 32 node-tiles
NPAIR = N // 256  # 16 edge pairs (256 nodes / 4096 edges each)
NQUAD = N // 512  # 8 quads
NEDGE_TOT = N_CORES * N * K  # 524288 (global BN count)
EPS = 1e-5
F32 = mybir.dt.float32
I16 = mybir.dt.int16
U32 = mybir.dt.uint32
ALU = mybir.AluOpType
AF = mybir.ActivationFunctionType
AX = mybir.AxisListType
NEG_BIG = -3.0e38


def _repl_const(kind):
    """Replication matrices for wrapped-index construction.

    idxT rows q = 16*j + k hold (row-tile j-in-quad, neighbor k).
    u-pair X (X in 0,1): out[p, e] uses idxT row 16*(2X + p//64) + p%16.
    e-quad: out[p, e] uses idxT row 16*(p//32) + p%16.
    """
    cols = 64 if kind in ("eA", "eB") else 128
    r = np.zeros((64, cols), np.float32)
    for q in range(64):
        j, kk = q // 16, q % 16
        for p in range(cols):
            if p % 16 != kk:
                continue
            if kind == "uA" and j == p // 64:
                r[q, p] = 1.0
            elif kind == "uB" and j == 2 + p // 64:
                r[q, p] = 1.0
            elif kind == "eA" and j == p // 32:
                r[q, p] = 1.0
            elif kind == "eB" and j == 2 + p // 32:
                r[q, p] = 1.0
    return r


def build():
    nc = bacc.Bacc("TRN2", target_bir_lowering=False, debug=False,
                   enable_asserts=False, num_devices=N_CORES)

    pos = nc.dram_tensor("pos", [N, PIN], F32, kind="ExternalInput")
    ene = nc.dram_tensor("ene", [N, EIN], F32, kind="ExternalInput")
    wx1 = nc.dram_tensor("wx1", [PIN, PIN], F32, kind="ExternalInput")
    wx2 = nc.dram_tensor("wx2", [PIN, PIN], F32, kind="ExternalInput")
    wpx_aug = nc.dram_tensor("wpx_aug", [PIN + 1, PIN], F32, kind="ExternalInput")
    wte = nc.dram_tensor("wte", [EIN, EIN], F32, kind="ExternalInput")
    wpe_aug = nc.dram_tensor("wpe_aug", [EIN + 1, EIN], F32, kind="ExternalInput")
    gx = nc.dram_tensor("gx", [PIN, 1], F32, kind="ExternalInput")
    betax = nc.dram_tensor("betax", [PIN, 1], F32, kind="ExternalInput")
    pos_next = nc.dram_tensor("pos_next", [N, PIN], F32, kind="ExternalOutput")
    ene_next = nc.dram_tensor("ene_next", [N, EIN], F32, kind="ExternalOutput")

    ru_a = nc.inline_tensor(_repl_const("uA"), name="ru_a")
    ru_b = nc.inline_tensor(_repl_const("uB"), name="ru_b")
    re_a = nc.inline_tensor(_repl_const("eA"), name="re_a")
    re_b = nc.inline_tensor(_repl_const("eB"), name="re_b")

    with tile.TileContext(nc) as tc:
        with tc.tile_pool(name="singles", bufs=1) as sg, \
             tc.tile_pool(name="rot", bufs=2) as rot:
            # ---------- constants ----------
            ident = sg.tile([P, P], F32)
            make_identity(nc, ident[:])
            eyeneg = sg.tile([P, P], F32)
            nc.gpsimd.memset(eyeneg[:], 0.0)
            nc.gpsimd.affine_select(
                out=eyeneg[:], in_=eyeneg[:], compare_op=ALU.not_equal,
                fill=-1.0e9, base=0, pattern=[[-1, P]], channel_multiplier=1)
            rua_sb = sg.tile([64, P], F32)
            nc.sync.dma_start(rua_sb[:], ru_a.ap())
            rub_sb = sg.tile([64, P], F32)
            nc.sync.dma_start(rub_sb[:], ru_b.ap())
            rea_sb = sg.tile([64, 64], F32)
            nc.sync.dma_start(rea_sb[:], re_a.ap())
            reb_sb = sg.tile([64, 64], F32)
            nc.sync.dma_start(reb_sb[:], re_b.ap())

            # master ui index list (wrapped): value = 128*(p//64) + col
            pmi = sg.tile([P, 1], mybir.dt.int32)
            nc.gpsimd.iota(pmi[:], pattern=[[0, 1]], base=0, channel_multiplier=1)
            nc.vector.tensor_scalar(pmi[:], pmi[:], 64, None, op0=ALU.bitwise_and)
            pmf = sg.tile([P, 1], F32)
            nc.vector.tensor_copy(pmf[:], pmi[:])
            nc.vector.tensor_scalar_mul(pmf[:], pmf[:], 2.0)
            mui_f = sg.tile([P, P], F32)
            nc.gpsimd.iota(mui_f[:], pattern=[[1, P]], base=0, channel_multiplier=0,
                           allow_small_or_imprecise_dtypes=True)
            nc.vector.tensor_scalar(mui_f[:], mui_f[:], pmf[:], None, op0=ALU.add)
            master_ui = sg.tile([P, P], I16)
            nc.vector.tensor_copy(master_ui[:], mui_f[:])

            # ---------- weights ----------
            wx1_sb = sg.tile([PIN, PIN], F32)
            nc.sync.dma_start(wx1_sb[:], wx1[:])
            bd2 = sg.tile([P, P], F32)
            nc.vector.memset(bd2[:], 0.0)
            nc.sync.dma_start(bd2[0:64, 0:64], wx2[:])
            nc.sync.dma_start(bd2[64:128, 64:128], wx2[:])
            wpxa_sb = sg.tile([PIN + 1, PIN], F32)
            nc.sync.dma_start(wpxa_sb[:], wpx_aug[:])
            bd2e = sg.tile([64, 64], F32)
            nc.vector.memset(bd2e[:], 0.0)
            for g in range(2):
                nc.sync.dma_start(bd2e[32 * g:32 * g + 32, 32 * g:32 * g + 32], wte[:])
            wpea_sb = sg.tile([EIN + 1, EIN], F32)
            nc.sync.dma_start(wpea_sb[:], wpe_aug[:])
            gx_sb = sg.tile([PIN, 1], F32)
            nc.sync.dma_start(gx_sb[:], gx[:])
            betax_sb = sg.tile([PIN, 1], F32)
            nc.sync.dma_start(betax_sb[:], betax[:])

            # ---------- persistent big tables ----------
            ptaug1 = sg.tile([96, N], F32)     # rows 0..63 posT, 64.. ones
            ptaug2 = sg.tile([65, N], F32)     # rows 0..63 2*posT, 64 = -sq
            tu = sg.tile([P, N], F32)          # U^T duplicated 2x
            te2 = sg.tile([64, N], F32)        # e^T duplicated 2x
            etaug = sg.tile([64, N], F32)      # rows 0..31 e^T, 32.. ones
            nc.vector.memset(ptaug1[64:96, :], 1.0)
            nc.vector.memset(etaug[32:64, :], 1.0)

            uw_all = sg.tile([P, NPAIR * P], I16)   # wrapped neighbor lists (u)
            ew_all = sg.tile([64, NPAIR * P], I16)  # wrapped neighbor lists (e)
            stash = sg.tile([P, NT * K], F32)       # idx per row-tile as f32
            statsz = sg.tile([P, NPAIR], F32)
            statsz2 = sg.tile([P, NPAIR], F32)

            with tc.tile_pool(name="psA", bufs=1, space="PSUM") as psA:
                # ---------- phase A: transposes + tables ----------
                for t in range(NT):
                    prt = rot.tile([P, PIN], F32, tag="prt")
                    nc.sync.dma_start(prt[:], pos[t * P:(t + 1) * P, :])
                    pt_ps = psA.tile([PIN, P], F32, tag="small", bufs=2)
                    nc.tensor.transpose(pt_ps[:], prt[:], ident[:])
                    nc.scalar.activation(ptaug1[0:64, t * P:(t + 1) * P], pt_ps[:],
                                         AF.Copy)
                    nc.scalar.activation(ptaug2[0:64, t * P:(t + 1) * P], pt_ps[:],
                                         AF.Copy, scale=2.0)
                    sq_col = rot.tile([P, 1], F32, tag="sqcol")
                    sq_scr = rot.tile([P, PIN], F32, tag="sqscr")
                    nc.scalar.activation(sq_scr[:], prt[:], AF.Square,
                                         accum_out=sq_col[:])
                    sqT_ps = psA.tile([1, P], F32, tag="small", bufs=2)
                    nc.tensor.transpose(sqT_ps[:], sq_col[:], ident[:])
                    nc.scalar.activation(ptaug2[64:65, t * P:(t + 1) * P], sqT_ps[:],
                                         AF.Copy, scale=-1.0)
                    ert = rot.tile([P, EIN], F32, tag="ert")
                    nc.sync.dma_start(ert[:], ene[t * P:(t + 1) * P, :])
                    et_ps = psA.tile([32, P], F32, tag="small", bufs=2)
                    nc.tensor.transpose(et_ps[:], ert[:], ident[:])
                    nc.scalar.activation(te2[0:32, t * P:(t + 1) * P], et_ps[:],
                                         AF.Copy)
                nc.scalar.activation(etaug[0:32, :], te2[0:32, :], AF.Copy)
                nc.sync.dma_start(te2[32:64, :], te2[0:32, :])
                # U^T = Wx1^T posT, duplicated into both halves of tu
                for cb in range(N // 512):
                    ups = psA.tile([P, 512], F32, tag="vps", bufs=4)
                    nc.tensor.matmul(ups[0:64, :], wx1_sb[:],
                                     ptaug1[0:64, cb * 512:(cb + 1) * 512],
                                     start=True, stop=True)
                    nc.tensor.matmul(ups[64:128, :], wx1_sb[:],
                                     ptaug1[0:64, cb * 512:(cb + 1) * 512],
                                     start=True, stop=True)
                    nc.scalar.activation(tu[:, cb * 512:(cb + 1) * 512], ups[:],
                                         AF.Copy)

                # ---------- KNN with software-pipelined wraps/gathers/stats ----------
                def emit_stats(qq):
                    for h in range(2):
                        pr = 2 * qq + h
                        ujt = rot.tile([P, 2048], F32, tag="ujt", bufs=2,
                                       name=f"ujt_s1_{pr}")
                        uit = rot.tile([P, 2048], F32, tag="uit", bufs=2,
                                       name=f"uit_s1_{pr}")
                        nc.gpsimd.ap_gather(
                            ujt[:], tu[:], uw_all[:, pr * P:(pr + 1) * P],
                            channels=P, num_elems=N, d=1, num_idxs=2048)
                        nc.gpsimd.ap_gather(
                            uit[:], tu[:, pr * 256:(pr + 1) * 256], master_ui[:],
                            channels=P, num_elems=256, d=1, num_idxs=2048)
                        zt = rot.tile([P, 2048], F32, tag="row2", bufs=2,
                                      name=f"zt_s1_{pr}")
                        nc.vector.scalar_tensor_tensor(
                            zt[:], ujt[:], 1.0, uit[:], op0=ALU.mult,
                            op1=ALU.subtract, accum_out=statsz[:, pr:pr + 1])
                        nc.scalar.activation(uit[:], zt[:], AF.Square,
                                             accum_out=statsz2[:, pr:pr + 1])

                def emit_wraps(qq):
                    ixT_ps = psA.tile([64, P], F32, tag="small", bufs=2,
                                      name=f"ixtps_{qq}")
                    nc.tensor.transpose(ixT_ps[:],
                                        stash[:, 4 * qq * K:(4 * qq + 4) * K],
                                        ident[:])
                    ixT = rot.tile([64, P], F32, tag="ixT", bufs=1,
                                   name=f"ixt_{qq}")
                    nc.scalar.activation(ixT[:], ixT_ps[:], AF.Copy)
                    for nm, cst, dst in (
                            ("wa", rua_sb, uw_all[:, (2 * qq) * P:(2 * qq + 1) * P]),
                            ("wb", rub_sb,
                             uw_all[:, (2 * qq + 1) * P:(2 * qq + 2) * P])):
                        wps = psA.tile([P, P], F32, tag="small", bufs=2,
                                       name=f"{nm}_{qq}")
                        nc.tensor.matmul(wps[:], cst[:], ixT[:], start=True,
                                         stop=True)
                        nc.vector.tensor_copy(dst, wps[:])
                    for nm, cst, dst in (
                            ("we", rea_sb, ew_all[:, (2 * qq) * P:(2 * qq + 1) * P]),
                            ("wf", reb_sb,
                             ew_all[:, (2 * qq + 1) * P:(2 * qq + 2) * P])):
                        weps = psA.tile([64, P], F32, tag="small", bufs=2,
                                        name=f"{nm}_{qq}")
                        nc.tensor.matmul(weps[:], cst[:], ixT[:], start=True,
                                         stop=True)
                        nc.vector.tensor_copy(dst, weps[:])

                for q in range(NQUAD):
                    if q >= 3:
                        emit_stats(q - 3)
                    if q >= 2:
                        emit_wraps(q - 2)
                    for j in range(4):
                        r = 4 * q + j
                        row1 = rot.tile([P, N], F32, tag="row1", bufs=2)
                        for cb in range(N // 512):
                            vps = psA.tile([P, 512], F32, tag="vps", bufs=4)
                            nc.tensor.matmul(
                                vps[:], ptaug1[0:65, r * P:(r + 1) * P],
                                ptaug2[0:65, cb * 512:(cb + 1) * 512],
                                start=True, stop=True)
                            nc.scalar.activation(row1[:, cb * 512:(cb + 1) * 512],
                                                 vps[:], AF.Copy)
                        nc.vector.tensor_add(row1[:, r * P:(r + 1) * P],
                                             row1[:, r * P:(r + 1) * P], eyeneg[:])
                        v8a = rot.tile([P, 8], F32, tag="v8a")
                        v8b = rot.tile([P, 8], F32, tag="v8b")
                        i8 = rot.tile([P, K], U32, tag="i8")
                        row2 = rot.tile([P, N], F32, tag="row2", bufs=2)
                        nc.vector.max(out=v8a[:], in_=row1[:])
                        nc.vector.max_index(out=i8[:, 0:8], in_max=v8a[:],
                                            in_values=row1[:])
                        nc.vector.match_replace(out=row2[:], in_to_replace=v8a[:],
                                                in_values=row1[:], imm_value=NEG_BIG)
                        nc.vector.max(out=v8b[:], in_=row2[:])
                        nc.vector.max_index(out=i8[:, 8:16], in_max=v8b[:],
                                            in_values=row2[:])
                        nc.vector.tensor_copy(stash[:, r * K:(r + 1) * K], i8[:])
                for q in range(NQUAD - 2, NQUAD):
                    emit_wraps(q)
                for q in range(NQUAD - 3, NQUAD):
                    emit_stats(q)

                # energy branch per pair (2-pack over 64 partitions)
                for pr in range(NPAIR):
                    ejt = rot.tile([64, 2048], F32, tag="ejt", bufs=1)
                    nc.gpsimd.ap_gather(
                        ejt[:], te2[:], ew_all[:, pr * P:(pr + 1) * P],
                        channels=64, num_elems=N, d=1, num_idxs=2048)
                    emean = rot.tile([64, P], F32, tag="emean")
                    nc.vector.reduce_sum(
                        emean[:], ejt[:].rearrange("p (a b) -> p a b", b=K),
                        axis=AX.X)
                    ev = rot.tile([64, P], F32, tag="ev")
                    for g in range(2):
                        nc.vector.scalar_tensor_tensor(
                            ev[32 * g:32 * g + 32, :],
                            emean[32 * g:32 * g + 32, :], 1.0 / K,
                            te2[32 * g:32 * g + 32,
                                pr * 256 + g * P:pr * 256 + (g + 1) * P],
                            op0=ALU.mult, op1=ALU.subtract)
                    eps_ = psA.tile([64, P], F32, tag="eps")
                    nc.tensor.matmul(eps_[:], bd2e[:], ev[:], start=True,
                                     stop=False, skip_group_check=True)
                    for g in range(2):
                        nc.tensor.matmul(
                            eps_[32 * g:32 * g + 32, :], wpea_sb[:],
                            etaug[0:33, pr * 256 + g * P:pr * 256 + (g + 1) * P],
                            start=False, stop=(g == 1), skip_group_check=True)
                    esb = rot.tile([64, P], F32, tag="esb")
                    nc.scalar.activation(esb[:], eps_[:], AF.Copy)
                    eT_ps = psA.tile([P, 64], F32, tag="rTe")
                    nc.tensor.transpose(eT_ps[:], esb[:], ident[0:64, 0:64])
                    esb2 = rot.tile([P, 64], F32, tag="esb2")
                    nc.scalar.activation(esb2[:], eT_ps[:], AF.Copy)
                    for g in range(2):
                        nc.sync.dma_start(
                            ene_next[pr * 256 + g * P:pr * 256 + (g + 1) * P, :],
                            esb2[:, 32 * g:32 * g + 32])
                # ---------- global BN stats (collective) ----------
                ssum = sg.tile([P, 2], F32)
                nc.vector.reduce_sum(ssum[:, 0:1], statsz[:], axis=AX.X)
                nc.vector.reduce_sum(ssum[:, 1:2], statsz2[:], axis=AX.X)
                comb = sg.tile([64, 2], F32)
                hi = sg.tile([64, 2], F32)
                nc.sync.dma_start(hi[:], ssum[64:128, :])
                nc.vector.tensor_add(comb[:], ssum[0:64, :], hi[:])
            # psA released here

            with tc.tile_pool(name="dramp", bufs=1, space="DRAM") as dramp:
                cc_in = dramp.tile([64, 2], F32)
                cc_out = dramp.tile([64, 2], F32)
                nc.sync.dma_start(cc_in[:], comb[:])
                nc.gpsimd.collective_compute(
                    "AllReduce", ALU.add,
                    replica_groups=[list(range(N_CORES))],
                    ins=[cc_in.opt()], outs=[cc_out.opt()])
                gstat = sg.tile([64, 2], F32)
                nc.sync.dma_start(gstat[:], cc_out[:])

            # BN coefficients a (scale), c (bias), duplicated to both halves
            mmean = sg.tile([64, 1], F32)
            nc.vector.tensor_scalar_mul(mmean[:], gstat[:, 0:1], 1.0 / NEDGE_TOT)
            msq = sg.tile([64, 1], F32)
            nc.vector.tensor_scalar_mul(msq[:], gstat[:, 1:2], 1.0 / NEDGE_TOT)
            nvar = sg.tile([64, 1], F32)   # m^2 - E[z^2]  (negated variance)
            nc.vector.scalar_tensor_tensor(nvar[:], mmean[:], mmean[:], msq[:],
                                           op0=ALU.mult, op1=ALU.subtract)
            epst = sg.tile([64, 1], F32)
            nc.vector.memset(epst[:], EPS)
            sdev = sg.tile([64, 1], F32)
            nc.scalar.activation(sdev[:], nvar[:], AF.Sqrt, bias=epst[:], scale=-1.0)
            inv = sg.tile([64, 1], F32)
            nc.vector.reciprocal(inv[:], sdev[:])
            a_full = sg.tile([P, 1], F32)
            c_full = sg.tile([P, 1], F32)
            nc.vector.tensor_tensor(a_full[0:64, :], inv[:], gx_sb[:], op=ALU.mult)
            ma = sg.tile([64, 1], F32)
            nc.vector.tensor_tensor(ma[:], mmean[:], a_full[0:64, :], op=ALU.mult)
            nc.vector.tensor_tensor(c_full[0:64, :], betax_sb[:], ma[:],
                                    op=ALU.subtract)
            nc.sync.dma_start(a_full[64:128, :], a_full[0:64, :])
            nc.sync.dma_start(c_full[64:128, :], c_full[0:64, :])

            # ---------- stage 2 ----------
            with tc.tile_pool(name="psB", bufs=1, space="PSUM") as psB:
                for pr in range(NPAIR):
                    ujt = rot.tile([P, 2048], F32, tag="ujt", bufs=2)
                    uit = rot.tile([P, 2048], F32, tag="uit", bufs=2)
                    nc.gpsimd.ap_gather(
                        ujt[:], tu[:], uw_all[:, pr * P:(pr + 1) * P],
                        channels=P, num_elems=N, d=1, num_idxs=2048)
                    nc.gpsimd.ap_gather(
                        uit[:], tu[:, pr * 256:(pr + 1) * 256], master_ui[:],
                        channels=P, num_elems=256, d=1, num_idxs=2048)
                    zt = rot.tile([P, 2048], F32, tag="row2", bufs=2)
                    nc.vector.scalar_tensor_tensor(zt[:], ujt[:], 1.0, uit[:],
                                                   op0=ALU.mult, op1=ALU.subtract)
                    nc.scalar.activation(zt[:], zt[:], AF.Relu, bias=c_full[:],
                                         scale=a_full[:])
                    ypair = rot.tile([P, P], F32, tag="ypair")
                    for j in range(4):
                        h2ps = psB.tile([P, 512], F32, tag="h2", bufs=2)
                        nc.tensor.matmul(h2ps[:], bd2[:],
                                         zt[:, j * 512:(j + 1) * 512],
                                         start=True, stop=True)
                        nc.vector.reduce_max(
                            ypair[:, j * 32:(j + 1) * 32],
                            h2ps[:].rearrange("p (a b) -> p a b", b=K), axis=AX.X)
                    skps = psB.tile([P, P], F32, tag="skps")
                    nc.tensor.matmul(skps[0:64, :], wpxa_sb[:],
                                     ptaug1[0:65, pr * 256:pr * 256 + 128],
                                     start=True, stop=True)
                    nc.tensor.matmul(skps[64:128, :], wpxa_sb[:],
                                     ptaug1[0:65, pr * 256 + 128:(pr + 1) * 256],
                                     start=True, stop=True)
                    res = rot.tile([P, P], F32, tag="res")
                    nc.vector.tensor_add(res[:], ypair[:], skps[:])
                    rT_ps = psB.tile([P, P], F32, tag="rT")
                    nc.tensor.transpose(rT_ps[:], res[:], ident[:])
                    osb = rot.tile([P, P], F32, tag="osb")
                    nc.scalar.activation(osb[:], rT_ps[:], AF.Copy)
                    nc.sync.dma_start(pos_next[pr * 256:pr * 256 + 128, :],
                                      osb[:, 0:64])
                    nc.sync.dma_start(pos_next[pr * 256 + 128:(pr + 1) * 256, :],
                                      osb[:, 64:128])
    nc.compile()
    return nc


_NC_CACHE = None


def _get_nc():
    global _NC_CACHE
    if _NC_CACHE is None:
        _NC_CACHE = build()
    return _NC_CACHE


def _run(inputs, trace=False):
    pos_feat = np.ascontiguousarray(inputs["pos_feat"], dtype=np.float32)
    ene_feat = np.ascontiguousarray(inputs["ene_feat"], dtype=np.float32)
    wx1 = np.ascontiguousarray(inputs["Wx1"], dtype=np.float32)
    wx2 = np.ascontiguousarray(inputs["Wx2"], dtype=np.float32)
    wpx_aug = np.concatenate(
        [np.asarray(inputs["Wpx"], np.float32),
         (np.asarray(inputs["bpx"], np.float32)
          + np.asarray(inputs["bx2"], np.float32))[None, :]], axis=0)
    wte = np.ascontiguousarray(inputs["Wte"], dtype=np.float32)
    wpe_aug = np.concatenate(
        [np.asarray(inputs["Wpe"], np.float32),
         (np.asarray(inputs["bpe"], np.float32)
          + np.asarray(inputs["bte"], np.float32))[None, :]], axis=0)
    gx = np.ascontiguousarray(np.asarray(inputs["gx"], np.float32).reshape(PIN, 1))
    betax = np.ascontiguousarray(
        np.asarray(inputs["betax"], np.float32).reshape(PIN, 1))
    assert int(inputs.get("k", K)) == K

    nc = _get_nc()
    in_maps = []
    for c in range(N_CORES):
        in_maps.append({
            "pos": np.ascontiguousarray(pos_feat[c]),
            "ene": np.ascontiguousarray(ene_feat[c]),
            "wx1": wx1, "wx2": wx2, "wpx_aug": np.ascontiguousarray(wpx_aug),
            "wte": wte, "wpe_aug": np.ascontiguousarray(wpe_aug),
            "gx": gx, "betax": betax,
        })
    res = run_bass_kernel_spmd(nc, in_maps, core_ids=list(range(N_CORES)),
                               trace=trace)
    pos_out = np.stack([res.results[c]["pos_next"] for c in range(N_CORES)])
    ene_out = np.stack([res.results[c]["ene_next"] for c in range(N_CORES)])
    return (pos_out, ene_out), res


def kernel(**inputs):
    out, _ = _run(inputs, trace=False)
    return out


# revision 14
# speedup vs baseline: 2.0790x; 2.0790x over previous
# DualEdgeConv Trainium2 Bass kernel.
# Data-parallel over batch: 1 batch per NeuronCore (B=8, 8 cores).
# Per core: KNN (augmented matmul + MAX8/FIND_INDEX8/MATCH_REPLACE8 top-16),
# on-chip ap_gather edge gathers, global BN stats via AllReduce collective.
import sys

sys.path.insert(0, "/opt/trn_rl_repo")

import numpy as np

import concourse.bass as bass
import concourse.mybir as mybir
import concourse.tile as tile
from concourse import bacc
from concourse.bass_utils import run_bass_kernel_spmd
from concourse.masks import make_identity

N_CORES = 8
N = 4096          # nodes per batch
PIN = 64          # pos feature dim
EIN = 32          # ene feature dim
K = 16            # neighbors
P = 128           # partitions
NT = N // P       # 32 node-tiles
NPAIR = N // 256  # 16 edge pairs (256 nodes / 4096 edges each)
NQUAD = N // 512  # 8 quads
NEDGE_TOT = N_CORES * N * K  # 524288 (global BN count)
EPS = 1e-5
F32 = mybir.dt.float32
I16 = mybir.dt.int16
U32 = mybir.dt.uint32
ALU = mybir.AluOpType
AF = mybir.ActivationFunctionType
AX = mybir.AxisListType
NEG_BIG = -3.0e38


def _repl_const(kind):
    """Replication matrices for wrapped-index construction.

    idxT rows q = 16*j + k hold (row-tile j-in-quad, neighbor k).
    u-pair X (X in 0,1): out[p, e] uses idxT row 16*(2X + p//64) + p%16.
    e-quad: out[p, e] uses idxT row 16*(p//32) + p%16.
    """
    cols = 64 if kind in ("eA", "eB") else 128
    r = np.zeros((64, cols), np.float32)
    for q in range(64):
        j, kk = q // 16, q % 16
        for p in range(cols):
            if p % 16 != kk:
                continue
            if kind == "uA" and j == p // 64:
                r[q, p] = 1.0
            elif kind == "uB" and j == 2 + p // 64:
                r[q, p] = 1.0
            elif kind == "eA" and j == p // 32:
                r[q, p] = 1.0
            elif kind == "eB" and j == 2 + p // 32:
                r[q, p] = 1.0
    return r


def build():
    nc = bacc.Bacc("TRN2", target_bir_lowering=False, debug=False,
                   enable_asserts=False, num_devices=N_CORES)

    pos = nc.dram_tensor("pos", [N, PIN], F32, kind="ExternalInput")
    ene = nc.dram_tensor("ene", [N, EIN], F32, kind="ExternalInput")
    wx1 = nc.dram_tensor("wx1", [PIN, PIN], F32, kind="ExternalInput")
    wx2 = nc.dram_tensor("wx2", [PIN, PIN], F32, kind="ExternalInput")
    wpx_aug = nc.dram_tensor("wpx_aug", [PIN + 1, PIN], F32, kind="ExternalInput")
    wte = nc.dram_tensor("wte", [EIN, EIN], F32, kind="ExternalInput")
    wpe_aug = nc.dram_tensor("wpe_aug", [EIN + 1, EIN], F32, kind="ExternalInput")
    gx = nc.dram_tensor("gx", [PIN, 1], F32, kind="ExternalInput")
    betax = nc.dram_tensor("betax", [PIN, 1], F32, kind="ExternalInput")
    pos_next = nc.dram_tensor("pos_next", [N, PIN], F32, kind="ExternalOutput")
    ene_next = nc.dram_tensor("ene_next", [N, EIN], F32, kind="ExternalOutput")

    ru_a = nc.inline_tensor(_repl_const("uA"), name="ru_a")
    ru_b = nc.inline_tensor(_repl_const("uB"), name="ru_b")
    re_a = nc.inline_tensor(_repl_const("eA"), name="re_a")
    re_b = nc.inline_tensor(_repl_const("eB"), name="re_b")

    uj_spill = [nc.dram_tensor(f"ujsp_{pr}", [P, 2048], F32, kind="Internal")
                for pr in range(NPAIR)]

    with tile.TileContext(nc) as tc:
        with tc.tile_pool(name="singles", bufs=1) as sg, \
             tc.tile_pool(name="rot", bufs=2) as rot:
            # ---------- constants ----------
            ident = sg.tile([P, P], F32)
            make_identity(nc, ident[:])
            eyeneg = sg.tile([P, P], F32)
            nc.gpsimd.memset(eyeneg[:], 0.0)
            nc.gpsimd.affine_select(
                out=eyeneg[:], in_=eyeneg[:], compare_op=ALU.not_equal,
                fill=-1.0e9, base=0, pattern=[[-1, P]], channel_multiplier=1)
            rua_sb = sg.tile([64, P], F32)
            nc.sync.dma_start(rua_sb[:], ru_a.ap())
            rub_sb = sg.tile([64, P], F32)
            nc.sync.dma_start(rub_sb[:], ru_b.ap())
            rea_sb = sg.tile([64, 64], F32)
            nc.sync.dma_start(rea_sb[:], re_a.ap())
            reb_sb = sg.tile([64, 64], F32)
            nc.sync.dma_start(reb_sb[:], re_b.ap())

            # master ui index list (wrapped): value = 128*(p//64) + col
            pmi = sg.tile([P, 1], mybir.dt.int32)
            nc.gpsimd.iota(pmi[:], pattern=[[0, 1]], base=0, channel_multiplier=1)
            nc.vector.tensor_scalar(pmi[:], pmi[:], 64, None, op0=ALU.bitwise_and)
            pmf = sg.tile([P, 1], F32)
            nc.vector.tensor_copy(pmf[:], pmi[:])
            nc.vector.tensor_scalar_mul(pmf[:], pmf[:], 2.0)
            mui_f = sg.tile([P, P], F32)
            nc.gpsimd.iota(mui_f[:], pattern=[[1, P]], base=0, channel_multiplier=0,
                           allow_small_or_imprecise_dtypes=True)
            nc.vector.tensor_scalar(mui_f[:], mui_f[:], pmf[:], None, op0=ALU.add)
            master_ui = sg.tile([P, P], I16)
            nc.vector.tensor_copy(master_ui[:], mui_f[:])

            # ---------- weights ----------
            wx1_sb = sg.tile([PIN, PIN], F32)
            nc.sync.dma_start(wx1_sb[:], wx1[:])
            bd2 = sg.tile([P, P], F32)
            nc.vector.memset(bd2[:], 0.0)
            nc.sync.dma_start(bd2[0:64, 0:64], wx2[:])
            nc.sync.dma_start(bd2[64:128, 64:128], wx2[:])
            wpxa_sb = sg.tile([PIN + 1, PIN], F32)
            nc.sync.dma_start(wpxa_sb[:], wpx_aug[:])
            bd2e = sg.tile([64, 64], F32)
            nc.vector.memset(bd2e[:], 0.0)
            for g in range(2):
                nc.sync.dma_start(bd2e[32 * g:32 * g + 32, 32 * g:32 * g + 32], wte[:])
            wpea_sb = sg.tile([EIN + 1, EIN], F32)
            nc.sync.dma_start(wpea_sb[:], wpe_aug[:])
            gx_sb = sg.tile([PIN, 1], F32)
            nc.sync.dma_start(gx_sb[:], gx[:])
            betax_sb = sg.tile([PIN, 1], F32)
            nc.sync.dma_start(betax_sb[:], betax[:])

            # ---------- persistent big tables ----------
            ptaug1 = sg.tile([96, N], F32)     # rows 0..63 posT, 64.. ones
            ptaug2 = sg.tile([65, N], F32)     # rows 0..63 2*posT, 64 = -sq
            tu = sg.tile([P, N], F32)          # U^T duplicated 2x
            te2 = sg.tile([64, N], F32)        # e^T duplicated 2x
            etaug = sg.tile([64, N], F32)      # rows 0..31 e^T, 32.. ones
            nc.vector.memset(ptaug1[64:96, :], 1.0)
            nc.vector.memset(etaug[32:64, :], 1.0)

            uw_all = sg.tile([P, NPAIR * P], I16)   # wrapped neighbor lists (u)
            ew_all = sg.tile([64, NPAIR * P], I16)  # wrapped neighbor lists (e)
            stash = sg.tile([P, NT * K], F32)       # idx per row-tile as f32
            statsz = sg.tile([P, NPAIR], F32)
            statsz2 = sg.tile([P, NPAIR], F32)

            with tc.tile_pool(name="psA", bufs=1, space="PSUM") as psA:
                # ---------- phase A: transposes + tables ----------
                for t in range(NT):
                    prt = rot.tile([P, PIN], F32, tag="prt")
                    nc.sync.dma_start(prt[:], pos[t * P:(t + 1) * P, :])
                    pt_ps = psA.tile([PIN, P], F32, tag="small", bufs=2)
                    nc.tensor.transpose(pt_ps[:], prt[:], ident[:])
                    nc.scalar.activation(ptaug1[0:64, t * P:(t + 1) * P], pt_ps[:],
                                         AF.Copy)
                    nc.scalar.activation(ptaug2[0:64, t * P:(t + 1) * P], pt_ps[:],
                                         AF.Copy, scale=2.0)
                    sq_col = rot.tile([P, 1], F32, tag="sqcol")
                    sq_scr = rot.tile([P, PIN], F32, tag="sqscr")
                    nc.scalar.activation(sq_scr[:], prt[:], AF.Square,
                                         accum_out=sq_col[:])
                    sqT_ps = psA.tile([1, P], F32, tag="small", bufs=2)
                    nc.tensor.transpose(sqT_ps[:], sq_col[:], ident[:])
                    nc.scalar.activation(ptaug2[64:65, t * P:(t + 1) * P], sqT_ps[:],
                                         AF.Copy, scale=-1.0)
                    ert = rot.tile([P, EIN], F32, tag="ert")
                    nc.sync.dma_start(ert[:], ene[t * P:(t + 1) * P, :])
                    et_ps = psA.tile([32, P], F32, tag="small", bufs=2)
                    nc.tensor.transpose(et_ps[:], ert[:], ident[:])
                    nc.scalar.activation(te2[0:32, t * P:(t + 1) * P], et_ps[:],
                                         AF.Copy)
                nc.scalar.activation(etaug[0:32, :], te2[0:32, :], AF.Copy)
                nc.sync.dma_start(te2[32:64, :], te2[0:32, :])
                # U^T = Wx1^T posT, duplicated into both halves of tu
                for cb in range(N // 512):
                    ups = psA.tile([P, 512], F32, tag="vps", bufs=4)
                    nc.tensor.matmul(ups[0:64, :], wx1_sb[:],
                                     ptaug1[0:64, cb * 512:(cb + 1) * 512],
                                     start=True, stop=True)
                    nc.tensor.matmul(ups[64:128, :], wx1_sb[:],
                                     ptaug1[0:64, cb * 512:(cb + 1) * 512],
                                     start=True, stop=True)
                    nc.scalar.activation(tu[:, cb * 512:(cb + 1) * 512], ups[:],
                                         AF.Copy)

                # ---------- KNN with software-pipelined wraps/gathers/stats ----------
                def emit_uis(pr, nm):
                    # ui strip: [128,128] = U columns for this pair's nodes,
                    # chunk A nodes on partitions 0:64, chunk B on 64:128
                    uis = rot.tile([P, P], F32, tag="uis", bufs=4,
                                   name=f"uis_{nm}_{pr}")
                    nc.sync.dma_start(uis[0:64, :],
                                      tu[0:64, pr * 256:pr * 256 + 128])
                    nc.sync.dma_start(uis[64:128, :],
                                      tu[0:64, pr * 256 + 128:(pr + 1) * 256])
                    return uis

                def emit_stats(qq):
                    for h in range(2):
                        pr = 2 * qq + h
                        ujt = rot.tile([P, 2048], F32, tag="ujt", bufs=2,
                                       name=f"ujt_s1_{pr}")
                        nc.gpsimd.ap_gather(
                            ujt[:], tu[:], uw_all[:, pr * P:(pr + 1) * P],
                            channels=P, num_elems=N, d=1, num_idxs=2048)
                        uis = emit_uis(pr, "s1")
                        zt = rot.tile([P, 2048], F32, tag="row2", bufs=2,
                                      name=f"zt_s1_{pr}")
                        zsq = rot.tile([P, 2048], F32, tag="uit", bufs=1,
                                       name=f"zsq_s1_{pr}")
                        nc.vector.scalar_tensor_tensor(
                            zt[:].rearrange("p (n k) -> p n k", k=K),
                            ujt[:].rearrange("p (n k) -> p n k", k=K), 1.0,
                            uis[:].to_broadcast([P, P, K]),
                            op0=ALU.mult, op1=ALU.subtract,
                            accum_out=statsz[:, pr:pr + 1])
                        nc.scalar.activation(zsq[:], zt[:], AF.Square,
                                             accum_out=statsz2[:, pr:pr + 1])
                        nc.sync.dma_start(uj_spill[pr].ap(), ujt[:])

                def emit_wraps(qq):
                    ixT_ps = psA.tile([64, P], F32, tag="small", bufs=2,
                                      name=f"ixtps_{qq}")
                    nc.tensor.transpose(ixT_ps[:],
                                        stash[:, 4 * qq * K:(4 * qq + 4) * K],
                                        ident[:])
                    ixT = rot.tile([64, P], F32, tag="ixT", bufs=1,
                                   name=f"ixt_{qq}")
                    nc.scalar.activation(ixT[:], ixT_ps[:], AF.Copy)
                    for nm, cst, dst in (
                            ("wa", rua_sb, uw_all[:, (2 * qq) * P:(2 * qq + 1) * P]),
                            ("wb", rub_sb,
                             uw_all[:, (2 * qq + 1) * P:(2 * qq + 2) * P])):
                        wps = psA.tile([P, P], F32, tag="small", bufs=2,
                                       name=f"{nm}_{qq}")
                        nc.tensor.matmul(wps[:], cst[:], ixT[:], start=True,
                                         stop=True)
                        nc.vector.tensor_copy(dst, wps[:])
                    for nm, cst, dst in (
                            ("we", rea_sb, ew_all[:, (2 * qq) * P:(2 * qq + 1) * P]),
                            ("wf", reb_sb,
                             ew_all[:, (2 * qq + 1) * P:(2 * qq + 2) * P])):
                        weps = psA.tile([64, P], F32, tag="small", bufs=2,
                                        name=f"{nm}_{qq}")
                        nc.tensor.matmul(weps[:], cst[:], ixT[:], start=True,
                                         stop=True)
                        nc.vector.tensor_copy(dst, weps[:])

                for q in range(NQUAD):
                    if q >= 3:
                        emit_stats(q - 3)
                    if q >= 2:
                        emit_wraps(q - 2)
                    for j in range(4):
                        r = 4 * q + j
                        row1 = rot.tile([P, N], F32, tag="row1", bufs=2)
                        for cb in range(N // 512):
                            vps = psA.tile([P, 512], F32, tag="vps", bufs=4)
                            nc.tensor.matmul(
                                vps[:], ptaug1[0:65, r * P:(r + 1) * P],
                                ptaug2[0:65, cb * 512:(cb + 1) * 512],
                                start=True, stop=True)
                            nc.scalar.activation(row1[:, cb * 512:(cb + 1) * 512],
                                                 vps[:], AF.Copy)
                        nc.vector.tensor_add(row1[:, r * P:(r + 1) * P],
                                             row1[:, r * P:(r + 1) * P], eyeneg[:])
                        v8a = rot.tile([P, 8], F32, tag="v8a")
                        v8b = rot.tile([P, 8], F32, tag="v8b")
                        i8 = rot.tile([P, K], U32, tag="i8")
                        row2 = rot.tile([P, N], F32, tag="row2", bufs=2)
                        nc.vector.max(out=v8a[:], in_=row1[:])
                        nc.vector.max_index(out=i8[:, 0:8], in_max=v8a[:],
                                            in_values=row1[:])
                        nc.vector.match_replace(out=row2[:], in_to_replace=v8a[:],
                                                in_values=row1[:], imm_value=NEG_BIG)
                        nc.vector.max(out=v8b[:], in_=row2[:])
                        nc.vector.max_index(out=i8[:, 8:16], in_max=v8b[:],
                                            in_values=row2[:])
                        nc.vector.tensor_copy(stash[:, r * K:(r + 1) * K], i8[:])
                for q in range(NQUAD - 2, NQUAD):
                    emit_wraps(q)
                for q in range(NQUAD - 3, NQUAD):
                    emit_stats(q)

                # energy branch per pair (2-pack over 64 partitions)
                for pr in range(NPAIR):
                    ejt = rot.tile([64, 2048], F32, tag="ejt", bufs=1)
                    nc.gpsimd.ap_gather(
                        ejt[:], te2[:], ew_all[:, pr * P:(pr + 1) * P],
                        channels=64, num_elems=N, d=1, num_idxs=2048)
                    emean = rot.tile([64, P], F32, tag="emean")
                    nc.vector.reduce_sum(
                        emean[:], ejt[:].rearrange("p (a b) -> p a b", b=K),
                        axis=AX.X)
                    ev = rot.tile([64, P], F32, tag="ev")
                    for g in range(2):
                        nc.vector.scalar_tensor_tensor(
                            ev[32 * g:32 * g + 32, :],
                            emean[32 * g:32 * g + 32, :], 1.0 / K,
                            te2[32 * g:32 * g + 32,
                                pr * 256 + g * P:pr * 256 + (g + 1) * P],
                            op0=ALU.mult, op1=ALU.subtract)
                    eps_ = psA.tile([64, P], F32, tag="eps")
                    nc.tensor.matmul(eps_[:], bd2e[:], ev[:], start=True,
                                     stop=False, skip_group_check=True)
                    for g in range(2):
                        nc.tensor.matmul(
                            eps_[32 * g:32 * g + 32, :], wpea_sb[:],
                            etaug[0:33, pr * 256 + g * P:pr * 256 + (g + 1) * P],
                            start=False, stop=(g == 1), skip_group_check=True)
                    esb = rot.tile([64, P], F32, tag="esb")
                    nc.scalar.activation(esb[:], eps_[:], AF.Copy)
                    eT_ps = psA.tile([P, 64], F32, tag="rTe")
                    nc.tensor.transpose(eT_ps[:], esb[:], ident[0:64, 0:64])
                    esb2 = rot.tile([P, 64], F32, tag="esb2")
                    nc.scalar.activation(esb2[:], eT_ps[:], AF.Copy)
                    for g in range(2):
                        nc.sync.dma_start(
                            ene_next[pr * 256 + g * P:pr * 256 + (g + 1) * P, :],
                            esb2[:, 32 * g:32 * g + 32])
                # ---------- global BN stats (collective) ----------
                ssum = sg.tile([P, 2], F32)
                nc.vector.reduce_sum(ssum[:, 0:1], statsz[:], axis=AX.X)
                nc.vector.reduce_sum(ssum[:, 1:2], statsz2[:], axis=AX.X)
                comb = sg.tile([64, 2], F32)
                hi = sg.tile([64, 2], F32)
                nc.sync.dma_start(hi[:], ssum[64:128, :])
                nc.vector.tensor_add(comb[:], ssum[0:64, :], hi[:])
            # psA released here

            with tc.tile_pool(name="dramp", bufs=1, space="DRAM") as dramp:
                cc_in = dramp.tile([64, 2], F32)
                cc_out = dramp.tile([64, 2], F32)
                nc.sync.dma_start(cc_in[:], comb[:])
                nc.gpsimd.collective_compute(
                    "AllReduce", ALU.add,
                    replica_groups=[list(range(N_CORES))],
                    ins=[cc_in.opt()], outs=[cc_out.opt()])
                gstat = sg.tile([64, 2], F32)
                nc.sync.dma_start(gstat[:], cc_out[:])

            # BN coefficients a (scale), c (bias), duplicated to both halves
            mmean = sg.tile([64, 1], F32)
            nc.vector.tensor_scalar_mul(mmean[:], gstat[:, 0:1], 1.0 / NEDGE_TOT)
            msq = sg.tile([64, 1], F32)
            nc.vector.tensor_scalar_mul(msq[:], gstat[:, 1:2], 1.0 / NEDGE_TOT)
            nvar = sg.tile([64, 1], F32)   # m^2 - E[z^2]  (negated variance)
            nc.vector.scalar_tensor_tensor(nvar[:], mmean[:], mmean[:], msq[:],
                                           op0=ALU.mult, op1=ALU.subtract)
            epst = sg.tile([64, 1], F32)
            nc.vector.memset(epst[:], EPS)
            sdev = sg.tile([64, 1], F32)
            nc.scalar.activation(sdev[:], nvar[:], AF.Sqrt, bias=epst[:], scale=-1.0)
            inv = sg.tile([64, 1], F32)
            nc.vector.reciprocal(inv[:], sdev[:])
            a_full = sg.tile([P, 1], F32)
            c_full = sg.tile([P, 1], F32)
            nc.vector.tensor_tensor(a_full[0:64, :], inv[:], gx_sb[:], op=ALU.mult)
            ma = sg.tile([64, 1], F32)
            nc.vector.tensor_tensor(ma[:], mmean[:], a_full[0:64, :], op=ALU.mult)
            nc.vector.tensor_tensor(c_full[0:64, :], betax_sb[:], ma[:],
                                    op=ALU.subtract)
            nc.sync.dma_start(a_full[64:128, :], a_full[0:64, :])
            nc.sync.dma_start(c_full[64:128, :], c_full[0:64, :])

            # ---------- stage 2 ----------
            with tc.tile_pool(name="psB", bufs=1, space="PSUM") as psB:
                for pr in range(NPAIR):
                    ujt = rot.tile([P, 2048], F32, tag="ujt", bufs=2)
                    nc.sync.dma_start(ujt[:], uj_spill[pr].ap())
                    uis = emit_uis(pr, "s2")
                    zt = rot.tile([P, 2048], F32, tag="row2", bufs=2)
                    nc.vector.scalar_tensor_tensor(
                        zt[:].rearrange("p (n k) -> p n k", k=K),
                        ujt[:].rearrange("p (n k) -> p n k", k=K), 1.0,
                        uis[:].to_broadcast([P, P, K]),
                        op0=ALU.mult, op1=ALU.subtract)
                    nc.scalar.activation(zt[:], zt[:], AF.Relu, bias=c_full[:],
                                         scale=a_full[:])
                    ypair = rot.tile([P, P], F32, tag="ypair")
                    for j in range(4):
                        h2ps = psB.tile([P, 512], F32, tag="h2", bufs=2)
                        nc.tensor.matmul(h2ps[:], bd2[:],
                                         zt[:, j * 512:(j + 1) * 512],
                                         start=True, stop=True)
                        nc.vector.reduce_max(
                            ypair[:, j * 32:(j + 1) * 32],
                            h2ps[:].rearrange("p (a b) -> p a b", b=K), axis=AX.X)
                    skps = psB.tile([P, P], F32, tag="skps")
                    nc.tensor.matmul(skps[0:64, :], wpxa_sb[:],
                                     ptaug1[0:65, pr * 256:pr * 256 + 128],
                                     start=True, stop=True)
                    nc.tensor.matmul(skps[64:128, :], wpxa_sb[:],
                                     ptaug1[0:65, pr * 256 + 128:(pr + 1) * 256],
                                     start=True, stop=True)
                    res = rot.tile([P, P], F32, tag="res")
                    nc.vector.tensor_add(res[:], ypair[:], skps[:])
                    rT_ps = psB.tile([P, P], F32, tag="rT")
                    nc.tensor.transpose(rT_ps[:], res[:], ident[:])
                    osb = rot.tile([P, P], F32, tag="osb")
                    nc.scalar.activation(osb[:], rT_ps[:], AF.Copy)
                    nc.sync.dma_start(pos_next[pr * 256:pr * 256 + 128, :],
                                      osb[:, 0:64])
                    nc.sync.dma_start(pos_next[pr * 256 + 128:(pr + 1) * 256, :],
                                      osb[:, 64:128])
    nc.compile()
    return nc


_NC_CACHE = None


def _get_nc():
    global _NC_CACHE
    if _NC_CACHE is None:
        _NC_CACHE = build()
    return _NC_CACHE


def _run(inputs, trace=False):
    pos_feat = np.ascontiguousarray(inputs["pos_feat"], dtype=np.float32)
    ene_feat = np.ascontiguousarray(inputs["ene_feat"], dtype=np.float32)
    wx1 = np.ascontiguousarray(inputs["Wx1"], dtype=np.float32)
    wx2 = np.ascontiguousarray(inputs["Wx2"], dtype=np.float32)
    wpx_aug = np.concatenate(
        [np.asarray(inputs["Wpx"], np.float32),
         (np.asarray(inputs["bpx"], np.float32)
          + np.asarray(inputs["bx2"], np.float32))[None, :]], axis=0)
    wte = np.ascontiguousarray(inputs["Wte"], dtype=np.float32)
    wpe_aug = np.concatenate(
        [np.asarray(inputs["Wpe"], np.float32),
         (np.asarray(inputs["bpe"], np.float32)
          + np.asarray(inputs["bte"], np.float32))[None, :]], axis=0)
    gx = np.ascontiguousarray(np.asarray(inputs["gx"], np.float32).reshape(PIN, 1))
    betax = np.ascontiguousarray(
        np.asarray(inputs["betax"], np.float32).reshape(PIN, 1))
    assert int(inputs.get("k", K)) == K

    nc = _get_nc()
    in_maps = []
    for c in range(N_CORES):
        in_maps.append({
            "pos": np.ascontiguousarray(pos_feat[c]),
            "ene": np.ascontiguousarray(ene_feat[c]),
            "wx1": wx1, "wx2": wx2, "wpx_aug": np.ascontiguousarray(wpx_aug),
            "wte": wte, "wpe_aug": np.ascontiguousarray(wpe_aug),
            "gx": gx, "betax": betax,
        })
    res = run_bass_kernel_spmd(nc, in_maps, core_ids=list(range(N_CORES)),
                               trace=trace)
    pos_out = np.stack([res.results[c]["pos_next"] for c in range(N_CORES)])
    ene_out = np.stack([res.results[c]["ene_next"] for c in range(N_CORES)])
    return (pos_out, ene_out), res


def kernel(**inputs):
    out, _ = _run(inputs, trace=False)
    return out


# revision 21
# speedup vs baseline: 2.5496x; 1.2263x over previous
# DualEdgeConv Trainium2 Bass kernel.
# Data-parallel over batch: 1 batch per NeuronCore (B=8, 8 cores).
# Per core: KNN (augmented matmul + MAX8/FIND_INDEX8/MATCH_REPLACE8 top-16),
# on-chip ap_gather edge gathers, global BN stats via AllReduce collective.
import sys

sys.path.insert(0, "/opt/trn_rl_repo")

import numpy as np

import concourse.bass as bass
import concourse.mybir as mybir
import concourse.tile as tile
from concourse import bacc
from concourse.bass_utils import run_bass_kernel_spmd
from concourse.masks import make_identity

N_CORES = 8
N = 4096          # nodes per batch
PIN = 64          # pos feature dim
EIN = 32          # ene feature dim
K = 16            # neighbors
P = 128           # partitions
NT = N // P       # 32 node-tiles
NPAIR = N // 256  # 16 edge pairs (256 nodes / 4096 edges each)
NQUAD = N // 512  # 8 quads
NEDGE_TOT = N_CORES * N * K  # 524288 (global BN count)
EPS = 1e-5
F32 = mybir.dt.float32
I16 = mybir.dt.int16
U32 = mybir.dt.uint32
ALU = mybir.AluOpType
AF = mybir.ActivationFunctionType
AX = mybir.AxisListType
NEG_BIG = -3.0e38


def _repl_const(kind):
    """Replication matrices for wrapped-index construction.

    idxT rows q = 16*j + k hold (row-tile j-in-quad, neighbor k).
    u-pair X (X in 0,1): out[p, e] uses idxT row 16*(2X + p//64) + p%16.
    e-quad: out[p, e] uses idxT row 16*(p//32) + p%16.
    """
    cols = 128
    r = np.zeros((64, cols), np.float32)
    for q in range(64):
        j, kk = q // 16, q % 16
        for p in range(cols):
            if p % 16 != kk:
                continue
            if kind == "uA" and j == p // 64:
                r[q, p] = 1.0
            elif kind == "uB" and j == 2 + p // 64:
                r[q, p] = 1.0
            elif kind == "e4" and j == p // 32:
                r[q, p] = 1.0
    return r


def build():
    nc = bacc.Bacc("TRN2", target_bir_lowering=False, debug=False,
                   enable_asserts=False, num_devices=N_CORES)

    pos = nc.dram_tensor("pos", [N, PIN], F32, kind="ExternalInput")
    ene = nc.dram_tensor("ene", [N, EIN], F32, kind="ExternalInput")
    wx1 = nc.dram_tensor("wx1", [PIN, PIN], F32, kind="ExternalInput")
    wx2 = nc.dram_tensor("wx2", [PIN, PIN], F32, kind="ExternalInput")
    wpx_aug = nc.dram_tensor("wpx_aug", [PIN + 1, PIN], F32, kind="ExternalInput")
    wte = nc.dram_tensor("wte", [EIN, EIN], F32, kind="ExternalInput")
    wpe_aug = nc.dram_tensor("wpe_aug", [EIN + 1, EIN], F32, kind="ExternalInput")
    gx = nc.dram_tensor("gx", [PIN, 1], F32, kind="ExternalInput")
    betax = nc.dram_tensor("betax", [PIN, 1], F32, kind="ExternalInput")
    pos_next = nc.dram_tensor("pos_next", [N, PIN], F32, kind="ExternalOutput")
    ene_next = nc.dram_tensor("ene_next", [N, EIN], F32, kind="ExternalOutput")

    ru_a = nc.inline_tensor(_repl_const("uA"), name="ru_a")
    ru_b = nc.inline_tensor(_repl_const("uB"), name="ru_b")
    rq_e = nc.inline_tensor(_repl_const("e4"), name="rq_e")

    uj_spill = [nc.dram_tensor(f"ujsp_{pr}", [P, 2048], F32, kind="Internal")
                for pr in range(NPAIR)]

    with tile.TileContext(nc) as tc:
        with tc.tile_pool(name="singles", bufs=1) as sg, \
             tc.tile_pool(name="rot", bufs=2) as rot:
            # ---------- constants ----------
            ident = sg.tile([P, P], F32)
            make_identity(nc, ident[:])
            eyeneg = sg.tile([P, P], F32)
            nc.gpsimd.memset(eyeneg[:], 0.0)
            nc.gpsimd.affine_select(
                out=eyeneg[:], in_=eyeneg[:], compare_op=ALU.not_equal,
                fill=-1.0e9, base=0, pattern=[[-1, P]], channel_multiplier=1)
            rua_sb = sg.tile([64, P], F32)
            nc.sync.dma_start(rua_sb[:], ru_a.ap())
            rub_sb = sg.tile([64, P], F32)
            nc.sync.dma_start(rub_sb[:], ru_b.ap())
            rqe_sb = sg.tile([64, P], F32)
            nc.sync.dma_start(rqe_sb[:], rq_e.ap())

            # ---------- weights ----------
            wx1_sb = sg.tile([PIN, PIN], F32)
            nc.sync.dma_start(wx1_sb[:], wx1[:])
            bd2 = sg.tile([P, P], F32)
            nc.vector.memset(bd2[:], 0.0)
            nc.sync.dma_start(bd2[0:64, 0:64], wx2[:])
            nc.sync.dma_start(bd2[64:128, 64:128], wx2[:])
            wpxa_sb = sg.tile([PIN + 1, PIN], F32)
            nc.sync.dma_start(wpxa_sb[:], wpx_aug[:])
            wte_sb = sg.tile([EIN, EIN], F32)
            nc.sync.dma_start(wte_sb[:], wte[:])
            wpea_sb = sg.tile([EIN + 1, EIN], F32)
            nc.sync.dma_start(wpea_sb[:], wpe_aug[:])
            gx_sb = sg.tile([PIN, 1], F32)
            nc.sync.dma_start(gx_sb[:], gx[:])
            betax_sb = sg.tile([PIN, 1], F32)
            nc.sync.dma_start(betax_sb[:], betax[:])

            # ---------- persistent big tables ----------
            ptaug1 = sg.tile([96, N], F32)     # rows 0..63 posT, 64.. ones
            ptaug2 = sg.tile([65, N], F32)     # rows 0..63 2*posT, 64 = -sq
            tu = sg.tile([P, N], F32)          # U^T duplicated 2x
            tew = sg.tile([P, N], F32)         # (e@Wte)^T duplicated 4x
            etaug = sg.tile([64, N], F32)      # rows 0..31 e^T, 32.. ones
            nc.vector.memset(ptaug1[64:96, :], 1.0)
            nc.vector.memset(etaug[32:64, :], 1.0)

            uw_all = sg.tile([P, NPAIR * P], I16)   # wrapped neighbor lists (u)
            ew_all = sg.tile([P, NQUAD * P], I16)   # wrapped neighbor lists (e)
            stash = sg.tile([P, NT * K], F32)       # idx per row-tile as f32
            statsz = sg.tile([P, NPAIR], F32)
            statsz2 = sg.tile([P, NPAIR], F32)

            with tc.tile_pool(name="psA", bufs=1, space="PSUM") as psA:
                # ---------- phase A: transposes + tables ----------
                for t in range(NT):
                    prt = rot.tile([P, PIN], F32, tag="prt")
                    nc.sync.dma_start(prt[:], pos[t * P:(t + 1) * P, :])
                    pt_ps = psA.tile([PIN, P], F32, tag="small", bufs=2)
                    nc.tensor.transpose(pt_ps[:], prt[:], ident[:])
                    nc.scalar.activation(ptaug1[0:64, t * P:(t + 1) * P], pt_ps[:],
                                         AF.Copy)
                    nc.scalar.activation(ptaug2[0:64, t * P:(t + 1) * P], pt_ps[:],
                                         AF.Copy, scale=2.0)
                    sq_col = rot.tile([P, 1], F32, tag="sqcol")
                    sq_scr = rot.tile([P, PIN], F32, tag="sqscr")
                    nc.scalar.activation(sq_scr[:], prt[:], AF.Square,
                                         accum_out=sq_col[:])
                    sqT_ps = psA.tile([1, P], F32, tag="small", bufs=2)
                    nc.tensor.transpose(sqT_ps[:], sq_col[:], ident[:])
                    nc.scalar.activation(ptaug2[64:65, t * P:(t + 1) * P], sqT_ps[:],
                                         AF.Copy, scale=-1.0)
                    ert = rot.tile([P, EIN], F32, tag="ert")
                    nc.sync.dma_start(ert[:], ene[t * P:(t + 1) * P, :])
                    et_ps = psA.tile([32, P], F32, tag="small", bufs=2)
                    nc.tensor.transpose(et_ps[:], ert[:], ident[:])
                    nc.scalar.activation(etaug[0:32, t * P:(t + 1) * P], et_ps[:],
                                         AF.Copy)

                # U^T = Wx1^T posT, duplicated into both halves of tu
                for cb in range(N // 512):
                    ups = psA.tile([P, 512], F32, tag="vps", bufs=4)
                    nc.tensor.matmul(ups[0:64, :], wx1_sb[:],
                                     ptaug1[0:64, cb * 512:(cb + 1) * 512],
                                     start=True, stop=True)
                    nc.tensor.matmul(ups[64:128, :], wx1_sb[:],
                                     ptaug1[0:64, cb * 512:(cb + 1) * 512],
                                     start=True, stop=True)
                    nc.scalar.activation(tu[:, cb * 512:(cb + 1) * 512], ups[:],
                                         AF.Copy)
                    eups = psA.tile([32, 512], F32, tag="small", bufs=2,
                                    name=f"eups_{cb}")
                    nc.tensor.matmul(eups[:], wte_sb[:],
                                     etaug[0:32, cb * 512:(cb + 1) * 512],
                                     start=True, stop=True)
                    nc.scalar.activation(tew[0:32, cb * 512:(cb + 1) * 512],
                                         eups[:], AF.Copy)
                for gg in range(1, 4):
                    nc.sync.dma_start(tew[32 * gg:32 * gg + 32, :], tew[0:32, :])

                # ---------- KNN with software-pipelined wraps/gathers/stats ----------
                def emit_uis(pr, nm):
                    # ui strip: [128,128] = U columns for this pair's nodes,
                    # chunk A nodes on partitions 0:64, chunk B on 64:128
                    uis = rot.tile([P, P], F32, tag="uis", bufs=4,
                                   name=f"uis_{nm}_{pr}")
                    nc.sync.dma_start(uis[0:64, :],
                                      tu[0:64, pr * 256:pr * 256 + 128])
                    nc.sync.dma_start(uis[64:128, :],
                                      tu[0:64, pr * 256 + 128:(pr + 1) * 256])
                    return uis

                def emit_stats(qq):
                    for h in range(2):
                        pr = 2 * qq + h
                        ujt = rot.tile([P, 2048], F32, tag="ujt", bufs=2,
                                       name=f"ujt_s1_{pr}")
                        nc.gpsimd.ap_gather(
                            ujt[:], tu[:], uw_all[:, pr * P:(pr + 1) * P],
                            channels=P, num_elems=N, d=1, num_idxs=2048)
                        uis = emit_uis(pr, "s1")
                        zt = rot.tile([P, 2048], F32, tag="row2", bufs=2,
                                      name=f"zt_s1_{pr}")
                        zsq = rot.tile([P, 2048], F32, tag="uit", bufs=1,
                                       name=f"zsq_s1_{pr}")
                        nc.vector.scalar_tensor_tensor(
                            zt[:].rearrange("p (n k) -> p n k", k=K),
                            ujt[:].rearrange("p (n k) -> p n k", k=K), 1.0,
                            uis[:].to_broadcast([P, P, K]),
                            op0=ALU.mult, op1=ALU.subtract,
                            accum_out=statsz[:, pr:pr + 1])
                        nc.scalar.activation(zsq[:], zt[:], AF.Square,
                                             accum_out=statsz2[:, pr:pr + 1])
                        nc.sync.dma_start(uj_spill[pr].ap(), ujt[:])

                def emit_wraps(qq):
                    ixT_ps = psA.tile([64, P], F32, tag="small", bufs=2,
                                      name=f"ixtps_{qq}")
                    nc.tensor.transpose(ixT_ps[:],
                                        stash[:, 4 * qq * K:(4 * qq + 4) * K],
                                        ident[:])
                    ixT = rot.tile([64, P], F32, tag="ixT", bufs=1,
                                   name=f"ixt_{qq}")
                    nc.scalar.activation(ixT[:], ixT_ps[:], AF.Copy)
                    for nm, cst, dst in (
                            ("wa", rua_sb, uw_all[:, (2 * qq) * P:(2 * qq + 1) * P]),
                            ("wb", rub_sb,
                             uw_all[:, (2 * qq + 1) * P:(2 * qq + 2) * P])):
                        wps = psA.tile([P, P], F32, tag="small", bufs=2,
                                       name=f"{nm}_{qq}")
                        nc.tensor.matmul(wps[:], cst[:], ixT[:], start=True,
                                         stop=True)
                        nc.vector.tensor_copy(dst, wps[:])
                    weps = psA.tile([P, P], F32, tag="small", bufs=2,
                                    name=f"we_{qq}")
                    nc.tensor.matmul(weps[:], rqe_sb[:], ixT[:], start=True,
                                     stop=True)
                    nc.vector.tensor_copy(ew_all[:, qq * P:(qq + 1) * P], weps[:])

                def emit_egather(qq):
                    ejt = rot.tile([P, 2048], F32, tag="ejt", bufs=1,
                                   name=f"ejt_{qq}")
                    nc.gpsimd.ap_gather(
                        ejt[:], tew[:], ew_all[:, qq * P:(qq + 1) * P],
                        channels=P, num_elems=N, d=1, num_idxs=2048)
                    return ejt

                ejt_tiles = {}

                def emit_eproc(qq):
                    ejt = ejt_tiles.pop(qq)
                    esum = rot.tile([P, P], F32, tag="esum", bufs=2,
                                    name=f"esum_{qq}")
                    nc.vector.reduce_sum(
                        esum[:], ejt[:].rearrange("p (a b) -> p a b", b=K),
                        axis=AX.X)
                    ev = rot.tile([P, P], F32, tag="ev", bufs=2,
                                  name=f"ev_{qq}")
                    nc.vector.tensor_scalar_mul(ev[:], esum[:], 1.0 / K)
                    # skip slice (e @ (Wpe - Wte) + bte + bpe)^T for this quad
                    skq = psA.tile([32, 512], F32, tag="small", bufs=2,
                                   name=f"skq_{qq}")
                    nc.tensor.matmul(skq[:], wpea_sb[:],
                                     etaug[0:33, qq * 512:(qq + 1) * 512],
                                     start=True, stop=True)
                    skq_sb = rot.tile([32, 512], F32, tag="skqsb", bufs=2,
                                      name=f"skqsb_{qq}")
                    nc.scalar.activation(skq_sb[:], skq[:], AF.Copy)
                    # transpose ev and accumulate the 4 skip transposes
                    rT = psA.tile([P, P], F32, tag="erT", bufs=1,
                                  name=f"erT_{qq}")
                    nc.tensor.transpose(rT[:], ev[:], ident[:])
                    rT2 = psA.tile([P, P], F32, tag="erT2", bufs=1,
                                   name=f"erT2_{qq}")
                    for c in range(4):
                        nc.tensor.transpose(rT2[:, 32 * c:32 * c + 32],
                                            skq_sb[:, c * P:(c + 1) * P],
                                            ident[0:32, 0:32])
                    sk_sb = rot.tile([P, P], F32, tag="sksb", bufs=2,
                                     name=f"sksb_{qq}")
                    nc.scalar.activation(sk_sb[:], rT2[:], AF.Copy)
                    esb2 = rot.tile([P, P], F32, tag="esb2", bufs=2,
                                    name=f"esb2_{qq}")
                    nc.vector.tensor_add(esb2[:], rT[:], sk_sb[:])
                    for c in range(4):
                        nc.sync.dma_start(
                            ene_next[qq * 512 + c * P:qq * 512 + (c + 1) * P, :],
                            esb2[:, 32 * c:32 * c + 32])

                for q in range(NQUAD):
                    if q >= 3:
                        emit_stats(q - 3)
                        emit_eproc(q - 3)
                    if q >= 2:
                        emit_wraps(q - 2)
                        ejt_tiles[q - 2] = emit_egather(q - 2)
                    for j in range(4):
                        r = 4 * q + j
                        row1 = rot.tile([P, N], F32, tag="row1", bufs=2)
                        for cb in range(N // 512):
                            vps = psA.tile([P, 512], F32, tag="vps", bufs=4)
                            nc.tensor.matmul(
                                vps[:], ptaug1[0:65, r * P:(r + 1) * P],
                                ptaug2[0:65, cb * 512:(cb + 1) * 512],
                                start=True, stop=True)
                            nc.scalar.activation(row1[:, cb * 512:(cb + 1) * 512],
                                                 vps[:], AF.Copy)
                        nc.vector.tensor_add(row1[:, r * P:(r + 1) * P],
                                             row1[:, r * P:(r + 1) * P], eyeneg[:])
                        v8a = rot.tile([P, 8], F32, tag="v8a")
                        v8b = rot.tile([P, 8], F32, tag="v8b")
                        i8 = rot.tile([P, K], U32, tag="i8")
                        row2 = rot.tile([P, N], F32, tag="row2", bufs=2)
                        nc.vector.max(out=v8a[:], in_=row1[:])
                        nc.vector.max_index(out=i8[:, 0:8], in_max=v8a[:],
                                            in_values=row1[:])
                        nc.vector.match_replace(out=row2[:], in_to_replace=v8a[:],
                                                in_values=row1[:], imm_value=NEG_BIG)
                        nc.vector.max(out=v8b[:], in_=row2[:])
                        nc.vector.max_index(out=i8[:, 8:16], in_max=v8b[:],
                                            in_values=row2[:])
                        nc.vector.tensor_copy(stash[:, r * K:(r + 1) * K], i8[:])
                for q in range(NQUAD - 2, NQUAD):
                    emit_wraps(q)
                    ejt_tiles[q] = emit_egather(q)
                for q in range(NQUAD - 3, NQUAD):
                    emit_stats(q)
                    emit_eproc(q)

                # ---------- global BN stats (collective) ----------
                ssum = sg.tile([P, 2], F32)
                nc.vector.reduce_sum(ssum[:, 0:1], statsz[:], axis=AX.X)
                nc.vector.reduce_sum(ssum[:, 1:2], statsz2[:], axis=AX.X)
                comb = sg.tile([64, 2], F32)
                hi = sg.tile([64, 2], F32)
                nc.sync.dma_start(hi[:], ssum[64:128, :])
                nc.vector.tensor_add(comb[:], ssum[0:64, :], hi[:])
            # psA released here

            with tc.tile_pool(name="dramp", bufs=1, space="DRAM") as dramp:
                cc_in = dramp.tile([64, 2], F32)
                cc_out = dramp.tile([64, 2], F32)
                nc.sync.dma_start(cc_in[:], comb[:])
                nc.gpsimd.collective_compute(
                    "AllReduce", ALU.add,
                    replica_groups=[list(range(N_CORES))],
                    ins=[cc_in.opt()], outs=[cc_out.opt()])
                gstat = sg.tile([64, 2], F32)
                nc.sync.dma_start(gstat[:], cc_out[:])

            # BN coefficients a (scale), c (bias), duplicated to both halves
            mmean = sg.tile([64, 1], F32)
            nc.vector.tensor_scalar_mul(mmean[:], gstat[:, 0:1], 1.0 / NEDGE_TOT)
            msq = sg.tile([64, 1], F32)
            nc.vector.tensor_scalar_mul(msq[:], gstat[:, 1:2], 1.0 / NEDGE_TOT)
            nvar = sg.tile([64, 1], F32)   # m^2 - E[z^2]  (negated variance)
            nc.vector.scalar_tensor_tensor(nvar[:], mmean[:], mmean[:], msq[:],
                                           op0=ALU.mult, op1=ALU.subtract)
            epst = sg.tile([64, 1], F32)
            nc.vector.memset(epst[:], EPS)
            sdev = sg.tile([64, 1], F32)
            nc.scalar.activation(sdev[:], nvar[:], AF.Sqrt, bias=epst[:], scale=-1.0)
            inv = sg.tile([64, 1], F32)
            nc.vector.reciprocal(inv[:], sdev[:])
            a_full = sg.tile([P, 1], F32)
            c_full = sg.tile([P, 1], F32)
            nc.vector.tensor_tensor(a_full[0:64, :], inv[:], gx_sb[:], op=ALU.mult)
            ma = sg.tile([64, 1], F32)
            nc.vector.tensor_tensor(ma[:], mmean[:], a_full[0:64, :], op=ALU.mult)
            nc.vector.tensor_tensor(c_full[0:64, :], betax_sb[:], ma[:],
                                    op=ALU.subtract)
            nc.sync.dma_start(a_full[64:128, :], a_full[0:64, :])
            nc.sync.dma_start(c_full[64:128, :], c_full[0:64, :])

            # ---------- stage 2 ----------
            with tc.tile_pool(name="psB", bufs=1, space="PSUM") as psB:
                for pr in range(NPAIR):
                    ujt = rot.tile([P, 2048], F32, tag="ujt", bufs=2)
                    nc.sync.dma_start(ujt[:], uj_spill[pr].ap())
                    uis = emit_uis(pr, "s2")
                    zt = rot.tile([P, 2048], F32, tag="row2", bufs=2)
                    nc.vector.scalar_tensor_tensor(
                        zt[:].rearrange("p (n k) -> p n k", k=K),
                        ujt[:].rearrange("p (n k) -> p n k", k=K), 1.0,
                        uis[:].to_broadcast([P, P, K]),
                        op0=ALU.mult, op1=ALU.subtract)
                    nc.scalar.activation(zt[:], zt[:], AF.Relu, bias=c_full[:],
                                         scale=a_full[:])
                    ypair = rot.tile([P, P], F32, tag="ypair")
                    for j in range(4):
                        h2ps = psB.tile([P, 512], F32, tag="h2", bufs=2)
                        nc.tensor.matmul(h2ps[:], bd2[:],
                                         zt[:, j * 512:(j + 1) * 512],
                                         start=True, stop=True)
                        nc.vector.reduce_max(
                            ypair[:, j * 32:(j + 1) * 32],
                            h2ps[:].rearrange("p (a b) -> p a b", b=K), axis=AX.X)
                    skps = psB.tile([P, P], F32, tag="skps")
                    nc.tensor.matmul(skps[0:64, :], wpxa_sb[:],
                                     ptaug1[0:65, pr * 256:pr * 256 + 128],
                                     start=True, stop=True)
                    nc.tensor.matmul(skps[64:128, :], wpxa_sb[:],
                                     ptaug1[0:65, pr * 256 + 128:(pr + 1) * 256],
                                     start=True, stop=True)
                    res = rot.tile([P, P], F32, tag="res")
                    nc.vector.tensor_add(res[:], ypair[:], skps[:])
                    rT_ps = psB.tile([P, P], F32, tag="rT")
                    nc.tensor.transpose(rT_ps[:], res[:], ident[:])
                    osb = rot.tile([P, P], F32, tag="osb")
                    nc.scalar.activation(osb[:], rT_ps[:], AF.Copy)
                    nc.sync.dma_start(pos_next[pr * 256:pr * 256 + 128, :],
                                      osb[:, 0:64])
                    nc.sync.dma_start(pos_next[pr * 256 + 128:(pr + 1) * 256, :],
                                      osb[:, 64:128])
    nc.compile()
    return nc


_NC_CACHE = None


def _get_nc():
    global _NC_CACHE
    if _NC_CACHE is None:
        _NC_CACHE = build()
    return _NC_CACHE


def _run(inputs, trace=False):
    pos_feat = np.ascontiguousarray(inputs["pos_feat"], dtype=np.float32)
    ene_feat = np.ascontiguousarray(inputs["ene_feat"], dtype=np.float32)
    wx1 = np.ascontiguousarray(inputs["Wx1"], dtype=np.float32)
    wx2 = np.ascontiguousarray(inputs["Wx2"], dtype=np.float32)
    wpx_aug = np.concatenate(
        [np.asarray(inputs["Wpx"], np.float32),
         (np.asarray(inputs["bpx"], np.float32)
          + np.asarray(inputs["bx2"], np.float32))[None, :]], axis=0)
    wte = np.ascontiguousarray(inputs["Wte"], dtype=np.float32)
    wpe_aug = np.concatenate(
        [np.asarray(inputs["Wpe"], np.float32)
         - np.asarray(inputs["Wte"], np.float32),
         (np.asarray(inputs["bpe"], np.float32)
          + np.asarray(inputs["bte"], np.float32))[None, :]], axis=0)
    gx = np.ascontiguousarray(np.asarray(inputs["gx"], np.float32).reshape(PIN, 1))
    betax = np.ascontiguousarray(
        np.asarray(inputs["betax"], np.float32).reshape(PIN, 1))
    assert int(inputs.get("k", K)) == K

    nc = _get_nc()
    in_maps = []
    for c in range(N_CORES):
        in_maps.append({
            "pos": np.ascontiguousarray(pos_feat[c]),
            "ene": np.ascontiguousarray(ene_feat[c]),
            "wx1": wx1, "wx2": wx2, "wpx_aug": np.ascontiguousarray(wpx_aug),
            "wte": wte, "wpe_aug": np.ascontiguousarray(wpe_aug),
            "gx": gx, "betax": betax,
        })
    res = run_bass_kernel_spmd(nc, in_maps, core_ids=list(range(N_CORES)),
                               trace=trace)
    pos_out = np.stack([res.results[c]["pos_next"] for c in range(N_CORES)])
    ene_out = np.stack([res.results[c]["ene_next"] for c in range(N_CORES)])
    return (pos_out, ene_out), res


def kernel(**inputs):
    out, _ = _run(inputs, trace=False)
    return out


# revision 22
# speedup vs baseline: 2.5958x; 1.0181x over previous
# DualEdgeConv Trainium2 Bass kernel.
# Data-parallel over batch: 1 batch per NeuronCore (B=8, 8 cores).
# Per core: KNN (augmented matmul + MAX8/FIND_INDEX8/MATCH_REPLACE8 top-16),
# on-chip ap_gather edge gathers, global BN stats via AllReduce collective.
import sys

sys.path.insert(0, "/opt/trn_rl_repo")

import numpy as np

import concourse.bass as bass
import concourse.mybir as mybir
import concourse.tile as tile
from concourse import bacc
from concourse.bass_utils import run_bass_kernel_spmd
from concourse.masks import make_identity

N_CORES = 8
N = 4096          # nodes per batch
PIN = 64          # pos feature dim
EIN = 32          # ene feature dim
K = 16            # neighbors
P = 128           # partitions
NT = N // P       # 32 node-tiles
NPAIR = N // 256  # 16 edge pairs (256 nodes / 4096 edges each)
NQUAD = N // 512  # 8 quads
NEDGE_TOT = N_CORES * N * K  # 524288 (global BN count)
EPS = 1e-5
F32 = mybir.dt.float32
I16 = mybir.dt.int16
U32 = mybir.dt.uint32
ALU = mybir.AluOpType
AF = mybir.ActivationFunctionType
AX = mybir.AxisListType
NEG_BIG = -3.0e38


def _repl_const(kind):
    """Replication matrices for wrapped-index construction.

    idxT rows q = 16*j + k hold (row-tile j-in-quad, neighbor k).
    u-pair X (X in 0,1): out[p, e] uses idxT row 16*(2X + p//64) + p%16.
    e-quad: out[p, e] uses idxT row 16*(p//32) + p%16.
    """
    cols = 128
    r = np.zeros((64, cols), np.float32)
    for q in range(64):
        j, kk = q // 16, q % 16
        for p in range(cols):
            if p % 16 != kk:
                continue
            if kind == "uA" and j == p // 64:
                r[q, p] = 1.0
            elif kind == "uB" and j == 2 + p // 64:
                r[q, p] = 1.0
            elif kind == "e4" and j == p // 32:
                r[q, p] = 1.0
    return r


def build():
    nc = bacc.Bacc("TRN2", target_bir_lowering=False, debug=False,
                   enable_asserts=False, num_devices=N_CORES)

    pos = nc.dram_tensor("pos", [N, PIN], F32, kind="ExternalInput")
    ene = nc.dram_tensor("ene", [N, EIN], F32, kind="ExternalInput")
    wx1 = nc.dram_tensor("wx1", [PIN, PIN], F32, kind="ExternalInput")
    wx2 = nc.dram_tensor("wx2", [PIN, PIN], F32, kind="ExternalInput")
    wpx_aug = nc.dram_tensor("wpx_aug", [PIN + 1, PIN], F32, kind="ExternalInput")
    wte = nc.dram_tensor("wte", [EIN, EIN], F32, kind="ExternalInput")
    wpe_aug = nc.dram_tensor("wpe_aug", [EIN + 1, EIN], F32, kind="ExternalInput")
    gx = nc.dram_tensor("gx", [PIN, 1], F32, kind="ExternalInput")
    betax = nc.dram_tensor("betax", [PIN, 1], F32, kind="ExternalInput")
    pos_next = nc.dram_tensor("pos_next", [N, PIN], F32, kind="ExternalOutput")
    ene_next = nc.dram_tensor("ene_next", [N, EIN], F32, kind="ExternalOutput")

    ru_a = nc.inline_tensor(_repl_const("uA"), name="ru_a")
    ru_b = nc.inline_tensor(_repl_const("uB"), name="ru_b")
    rq_e = nc.inline_tensor(_repl_const("e4"), name="rq_e")

    uj_spill = [nc.dram_tensor(f"ujsp_{pr}", [P, 2048], F32, kind="Internal")
                for pr in range(NPAIR)]

    with tile.TileContext(nc) as tc:
        with tc.tile_pool(name="singles", bufs=1) as sg, \
             tc.tile_pool(name="rot", bufs=2) as rot:
            # ---------- constants ----------
            ident = sg.tile([P, P], F32)
            make_identity(nc, ident[:])
            eyeneg = sg.tile([P, P], F32)
            nc.gpsimd.memset(eyeneg[:], 0.0)
            nc.gpsimd.affine_select(
                out=eyeneg[:], in_=eyeneg[:], compare_op=ALU.not_equal,
                fill=-1.0e9, base=0, pattern=[[-1, P]], channel_multiplier=1)
            rua_sb = sg.tile([64, P], F32)
            nc.sync.dma_start(rua_sb[:], ru_a.ap())
            rub_sb = sg.tile([64, P], F32)
            nc.sync.dma_start(rub_sb[:], ru_b.ap())
            rqe_sb = sg.tile([64, P], F32)
            nc.sync.dma_start(rqe_sb[:], rq_e.ap())

            # ---------- weights ----------
            wx1_sb = sg.tile([PIN, PIN], F32)
            nc.sync.dma_start(wx1_sb[:], wx1[:])
            bd2 = sg.tile([P, P], F32)
            nc.vector.memset(bd2[:], 0.0)
            nc.sync.dma_start(bd2[0:64, 0:64], wx2[:])
            nc.sync.dma_start(bd2[64:128, 64:128], wx2[:])
            wpxa_sb = sg.tile([PIN + 1, PIN], F32)
            nc.sync.dma_start(wpxa_sb[:], wpx_aug[:])
            wte_sb = sg.tile([EIN, EIN], F32)
            nc.sync.dma_start(wte_sb[:], wte[:])
            wpea_sb = sg.tile([EIN + 1, EIN], F32)
            nc.sync.dma_start(wpea_sb[:], wpe_aug[:])
            gx_sb = sg.tile([PIN, 1], F32)
            nc.sync.dma_start(gx_sb[:], gx[:])
            betax_sb = sg.tile([PIN, 1], F32)
            nc.sync.dma_start(betax_sb[:], betax[:])

            # ---------- persistent big tables ----------
            ptaug1 = sg.tile([96, N], F32)     # rows 0..63 posT, 64.. ones
            ptaug2 = sg.tile([65, N], F32)     # rows 0..63 2*posT, 64 = -sq
            tu = sg.tile([P, N], F32)          # U^T duplicated 2x
            tew = sg.tile([P, N], F32)         # (e@Wte)^T duplicated 4x
            etaug = sg.tile([64, N], F32)      # rows 0..31 e^T, 32.. ones
            nc.vector.memset(ptaug1[64:96, :], 1.0)
            nc.vector.memset(etaug[32:64, :], 1.0)

            uw_all = sg.tile([P, NPAIR * P], I16)   # wrapped neighbor lists (u)
            ew_all = sg.tile([P, NQUAD * P], I16)   # wrapped neighbor lists (e)
            stash = sg.tile([P, NT * K], F32)       # idx per row-tile as f32
            statsz = sg.tile([P, NPAIR], F32)
            statsz2 = sg.tile([P, NPAIR], F32)

            with tc.tile_pool(name="psA", bufs=1, space="PSUM") as psA:
                # ---------- phase A: transposes + tables ----------
                for t in range(NT):
                    prt = rot.tile([P, PIN], F32, tag="prt")
                    nc.sync.dma_start(prt[:], pos[t * P:(t + 1) * P, :])
                    pt_ps = psA.tile([PIN, P], F32, tag="small", bufs=2)
                    nc.tensor.transpose(pt_ps[:], prt[:], ident[:])
                    nc.scalar.activation(ptaug1[0:64, t * P:(t + 1) * P], pt_ps[:],
                                         AF.Copy)
                    nc.scalar.activation(ptaug2[0:64, t * P:(t + 1) * P], pt_ps[:],
                                         AF.Copy, scale=2.0)
                    sq_col = rot.tile([P, 1], F32, tag="sqcol")
                    sq_scr = rot.tile([P, PIN], F32, tag="sqscr")
                    nc.scalar.activation(sq_scr[:], prt[:], AF.Square,
                                         accum_out=sq_col[:])
                    sqT_ps = psA.tile([1, P], F32, tag="small", bufs=2)
                    nc.tensor.transpose(sqT_ps[:], sq_col[:], ident[:])
                    nc.scalar.activation(ptaug2[64:65, t * P:(t + 1) * P], sqT_ps[:],
                                         AF.Copy, scale=-1.0)
                    ert = rot.tile([P, EIN], F32, tag="ert")
                    nc.sync.dma_start(ert[:], ene[t * P:(t + 1) * P, :])
                    et_ps = psA.tile([32, P], F32, tag="small", bufs=2)
                    nc.tensor.transpose(et_ps[:], ert[:], ident[:])
                    nc.scalar.activation(etaug[0:32, t * P:(t + 1) * P], et_ps[:],
                                         AF.Copy)

                # U^T = Wx1^T posT, duplicated into both halves of tu
                for cb in range(N // 512):
                    ups = psA.tile([P, 512], F32, tag="vps", bufs=4)
                    nc.tensor.matmul(ups[0:64, :], wx1_sb[:],
                                     ptaug1[0:64, cb * 512:(cb + 1) * 512],
                                     start=True, stop=True)
                    nc.tensor.matmul(ups[64:128, :], wx1_sb[:],
                                     ptaug1[0:64, cb * 512:(cb + 1) * 512],
                                     start=True, stop=True)
                    nc.scalar.activation(tu[:, cb * 512:(cb + 1) * 512], ups[:],
                                         AF.Copy)
                    eups = psA.tile([32, 512], F32, tag="small", bufs=2,
                                    name=f"eups_{cb}")
                    nc.tensor.matmul(eups[:], wte_sb[:],
                                     etaug[0:32, cb * 512:(cb + 1) * 512],
                                     start=True, stop=True)
                    nc.scalar.activation(tew[0:32, cb * 512:(cb + 1) * 512],
                                         eups[:], AF.Copy)
                for gg in range(1, 4):
                    nc.sync.dma_start(tew[32 * gg:32 * gg + 32, :], tew[0:32, :])

                # ---------- KNN with software-pipelined wraps/gathers/stats ----------
                def emit_uis(pr, nm):
                    # ui strip: [128,128] = U columns for this pair's nodes,
                    # chunk A nodes on partitions 0:64, chunk B on 64:128
                    uis = rot.tile([P, P], F32, tag="uis", bufs=4,
                                   name=f"uis_{nm}_{pr}")
                    nc.sync.dma_start(uis[0:64, :],
                                      tu[0:64, pr * 256:pr * 256 + 128])
                    nc.sync.dma_start(uis[64:128, :],
                                      tu[0:64, pr * 256 + 128:(pr + 1) * 256])
                    return uis

                def emit_ugather(qq):
                    for h in range(2):
                        pr = 2 * qq + h
                        ujt = rot.tile([P, 2048], F32, tag="ujt", bufs=2,
                                       name=f"ujt_s1_{pr}")
                        nc.gpsimd.ap_gather(
                            ujt[:], tu[:], uw_all[:, pr * P:(pr + 1) * P],
                            channels=P, num_elems=N, d=1, num_idxs=2048)
                        nc.sync.dma_start(uj_spill[pr].ap(), ujt[:])

                def emit_ureload(qq):
                    tiles = []
                    for h in range(2):
                        pr = 2 * qq + h
                        ujr = rot.tile([P, 2048], F32, tag="ujt", bufs=2,
                                       name=f"ujr_{pr}")
                        nc.sync.dma_start(ujr[:], uj_spill[pr].ap())
                        tiles.append(ujr)
                    return tiles

                ujr_tiles = {}

                def emit_ustats(qq):
                    for h in range(2):
                        pr = 2 * qq + h
                        ujr = ujr_tiles[qq][h]
                        uis = emit_uis(pr, "s1")
                        zt = rot.tile([P, 2048], F32, tag="row2", bufs=2,
                                      name=f"zt_s1_{pr}")
                        nc.vector.scalar_tensor_tensor(
                            zt[:].rearrange("p (n k) -> p n k", k=K),
                            ujr[:].rearrange("p (n k) -> p n k", k=K), 1.0,
                            uis[:].to_broadcast([P, P, K]),
                            op0=ALU.mult, op1=ALU.subtract,
                            accum_out=statsz[:, pr:pr + 1])
                        nc.scalar.activation(ujr[:], zt[:], AF.Square,
                                             accum_out=statsz2[:, pr:pr + 1])
                    del ujr_tiles[qq]

                def emit_wraps(qq):
                    ixT_ps = psA.tile([64, P], F32, tag="small", bufs=2,
                                      name=f"ixtps_{qq}")
                    nc.tensor.transpose(ixT_ps[:],
                                        stash[:, 4 * qq * K:(4 * qq + 4) * K],
                                        ident[:])
                    ixT = rot.tile([64, P], F32, tag="ixT", bufs=1,
                                   name=f"ixt_{qq}")
                    nc.scalar.activation(ixT[:], ixT_ps[:], AF.Copy)
                    for nm, cst, dst in (
                            ("wa", rua_sb, uw_all[:, (2 * qq) * P:(2 * qq + 1) * P]),
                            ("wb", rub_sb,
                             uw_all[:, (2 * qq + 1) * P:(2 * qq + 2) * P])):
                        wps = psA.tile([P, P], F32, tag="small", bufs=2,
                                       name=f"{nm}_{qq}")
                        nc.tensor.matmul(wps[:], cst[:], ixT[:], start=True,
                                         stop=True)
                        nc.vector.tensor_copy(dst, wps[:])
                    weps = psA.tile([P, P], F32, tag="small", bufs=2,
                                    name=f"we_{qq}")
                    nc.tensor.matmul(weps[:], rqe_sb[:], ixT[:], start=True,
                                     stop=True)
                    nc.vector.tensor_copy(ew_all[:, qq * P:(qq + 1) * P], weps[:])

                def emit_egather(qq):
                    ejt = rot.tile([P, 2048], F32, tag="ejt", bufs=2,
                                   name=f"ejt_{qq}")
                    nc.gpsimd.ap_gather(
                        ejt[:], tew[:], ew_all[:, qq * P:(qq + 1) * P],
                        channels=P, num_elems=N, d=1, num_idxs=2048)
                    return ejt

                ejt_tiles = {}

                def emit_eproc(qq):
                    ejt = ejt_tiles.pop(qq)
                    esum = rot.tile([P, P], F32, tag="esum", bufs=2,
                                    name=f"esum_{qq}")
                    nc.vector.reduce_sum(
                        esum[:], ejt[:].rearrange("p (a b) -> p a b", b=K),
                        axis=AX.X)
                    ev = rot.tile([P, P], F32, tag="ev", bufs=2,
                                  name=f"ev_{qq}")
                    nc.vector.tensor_scalar_mul(ev[:], esum[:], 1.0 / K)
                    # skip slice (e @ (Wpe - Wte) + bte + bpe)^T for this quad
                    skq = psA.tile([32, 512], F32, tag="small", bufs=2,
                                   name=f"skq_{qq}")
                    nc.tensor.matmul(skq[:], wpea_sb[:],
                                     etaug[0:33, qq * 512:(qq + 1) * 512],
                                     start=True, stop=True)
                    skq_sb = rot.tile([32, 512], F32, tag="skqsb", bufs=2,
                                      name=f"skqsb_{qq}")
                    nc.scalar.activation(skq_sb[:], skq[:], AF.Copy)
                    # transpose ev and accumulate the 4 skip transposes
                    rT = psA.tile([P, P], F32, tag="erT", bufs=1,
                                  name=f"erT_{qq}")
                    nc.tensor.transpose(rT[:], ev[:], ident[:])
                    rT2 = psA.tile([P, P], F32, tag="erT2", bufs=1,
                                   name=f"erT2_{qq}")
                    for c in range(4):
                        nc.tensor.transpose(rT2[:, 32 * c:32 * c + 32],
                                            skq_sb[:, c * P:(c + 1) * P],
                                            ident[0:32, 0:32])
                    sk_sb = rot.tile([P, P], F32, tag="sksb", bufs=2,
                                     name=f"sksb_{qq}")
                    nc.scalar.activation(sk_sb[:], rT2[:], AF.Copy)
                    esb2 = rot.tile([P, P], F32, tag="esb2", bufs=2,
                                    name=f"esb2_{qq}")
                    nc.vector.tensor_add(esb2[:], rT[:], sk_sb[:])
                    for c in range(4):
                        nc.sync.dma_start(
                            ene_next[qq * 512 + c * P:qq * 512 + (c + 1) * P, :],
                            esb2[:, 32 * c:32 * c + 32])

                for q in range(NQUAD):
                    if q >= 3:
                        ujr_tiles[q - 3] = emit_ureload(q - 3)
                    for j in range(4):
                        if j == 1 and q >= 1:
                            emit_wraps(q - 1)
                            emit_ugather(q - 1)
                            ejt_tiles[q - 1] = emit_egather(q - 1)
                        r = 4 * q + j
                        row1 = rot.tile([P, N], F32, tag="row1", bufs=2)
                        for cb in range(N // 512):
                            vps = psA.tile([P, 512], F32, tag="vps", bufs=4)
                            nc.tensor.matmul(
                                vps[:], ptaug1[0:65, r * P:(r + 1) * P],
                                ptaug2[0:65, cb * 512:(cb + 1) * 512],
                                start=True, stop=True)
                            nc.scalar.activation(row1[:, cb * 512:(cb + 1) * 512],
                                                 vps[:], AF.Copy)
                        nc.vector.tensor_add(row1[:, r * P:(r + 1) * P],
                                             row1[:, r * P:(r + 1) * P], eyeneg[:])
                        v8a = rot.tile([P, 8], F32, tag="v8a")
                        v8b = rot.tile([P, 8], F32, tag="v8b")
                        i8 = rot.tile([P, K], U32, tag="i8")
                        row2 = rot.tile([P, N], F32, tag="row2", bufs=2)
                        nc.vector.max(out=v8a[:], in_=row1[:])
                        nc.vector.max_index(out=i8[:, 0:8], in_max=v8a[:],
                                            in_values=row1[:])
                        nc.vector.match_replace(out=row2[:], in_to_replace=v8a[:],
                                                in_values=row1[:], imm_value=NEG_BIG)
                        nc.vector.max(out=v8b[:], in_=row2[:])
                        nc.vector.max_index(out=i8[:, 8:16], in_max=v8b[:],
                                            in_values=row2[:])
                        nc.vector.tensor_copy(stash[:, r * K:(r + 1) * K], i8[:])
                    if q >= 3:
                        emit_ustats(q - 3)
                        emit_eproc(q - 3)
                emit_wraps(NQUAD - 1)
                emit_ugather(NQUAD - 1)
                ejt_tiles[NQUAD - 1] = emit_egather(NQUAD - 1)
                for q in range(NQUAD - 3, NQUAD):
                    ujr_tiles[q] = emit_ureload(q)
                    emit_ustats(q)
                    emit_eproc(q)

                # ---------- global BN stats (collective) ----------
                ssum = sg.tile([P, 2], F32)
                nc.vector.reduce_sum(ssum[:, 0:1], statsz[:], axis=AX.X)
                nc.vector.reduce_sum(ssum[:, 1:2], statsz2[:], axis=AX.X)
                comb = sg.tile([64, 2], F32)
                hi = sg.tile([64, 2], F32)
                nc.sync.dma_start(hi[:], ssum[64:128, :])
                nc.vector.tensor_add(comb[:], ssum[0:64, :], hi[:])
            # psA released here

            with tc.tile_pool(name="dramp", bufs=1, space="DRAM") as dramp:
                cc_in = dramp.tile([64, 2], F32)
                cc_out = dramp.tile([64, 2], F32)
                nc.sync.dma_start(cc_in[:], comb[:])
                nc.gpsimd.collective_compute(
                    "AllReduce", ALU.add,
                    replica_groups=[list(range(N_CORES))],
                    ins=[cc_in.opt()], outs=[cc_out.opt()])
                gstat = sg.tile([64, 2], F32)
                nc.sync.dma_start(gstat[:], cc_out[:])

            # BN coefficients a (scale), c (bias), duplicated to both halves
            mmean = sg.tile([64, 1], F32)
            nc.vector.tensor_scalar_mul(mmean[:], gstat[:, 0:1], 1.0 / NEDGE_TOT)
            msq = sg.tile([64, 1], F32)
            nc.vector.tensor_scalar_mul(msq[:], gstat[:, 1:2], 1.0 / NEDGE_TOT)
            nvar = sg.tile([64, 1], F32)   # m^2 - E[z^2]  (negated variance)
            nc.vector.scalar_tensor_tensor(nvar[:], mmean[:], mmean[:], msq[:],
                                           op0=ALU.mult, op1=ALU.subtract)
            epst = sg.tile([64, 1], F32)
            nc.vector.memset(epst[:], EPS)
            sdev = sg.tile([64, 1], F32)
            nc.scalar.activation(sdev[:], nvar[:], AF.Sqrt, bias=epst[:], scale=-1.0)
            inv = sg.tile([64, 1], F32)
            nc.vector.reciprocal(inv[:], sdev[:])
            a_full = sg.tile([P, 1], F32)
            c_full = sg.tile([P, 1], F32)
            nc.vector.tensor_tensor(a_full[0:64, :], inv[:], gx_sb[:], op=ALU.mult)
            ma = sg.tile([64, 1], F32)
            nc.vector.tensor_tensor(ma[:], mmean[:], a_full[0:64, :], op=ALU.mult)
            nc.vector.tensor_tensor(c_full[0:64, :], betax_sb[:], ma[:],
                                    op=ALU.subtract)
            nc.sync.dma_start(a_full[64:128, :], a_full[0:64, :])
            nc.sync.dma_start(c_full[64:128, :], c_full[0:64, :])

            # ---------- stage 2 ----------
            with tc.tile_pool(name="psB", bufs=1, space="PSUM") as psB:
                for pr in range(NPAIR):
                    ujt = rot.tile([P, 2048], F32, tag="ujt", bufs=2)
                    nc.sync.dma_start(ujt[:], uj_spill[pr].ap())
                    uis = emit_uis(pr, "s2")
                    zt = rot.tile([P, 2048], F32, tag="row2", bufs=2)
                    nc.vector.scalar_tensor_tensor(
                        zt[:].rearrange("p (n k) -> p n k", k=K),
                        ujt[:].rearrange("p (n k) -> p n k", k=K), 1.0,
                        uis[:].to_broadcast([P, P, K]),
                        op0=ALU.mult, op1=ALU.subtract)
                    nc.scalar.activation(zt[:], zt[:], AF.Relu, bias=c_full[:],
                                         scale=a_full[:])
                    ypair = rot.tile([P, P], F32, tag="ypair")
                    for j in range(4):
                        h2ps = psB.tile([P, 512], F32, tag="h2", bufs=2)
                        nc.tensor.matmul(h2ps[:], bd2[:],
                                         zt[:, j * 512:(j + 1) * 512],
                                         start=True, stop=True)
                        nc.vector.reduce_max(
                            ypair[:, j * 32:(j + 1) * 32],
                            h2ps[:].rearrange("p (a b) -> p a b", b=K), axis=AX.X)
                    skps = psB.tile([P, P], F32, tag="skps")
                    nc.tensor.matmul(skps[0:64, :], wpxa_sb[:],
                                     ptaug1[0:65, pr * 256:pr * 256 + 128],
                                     start=True, stop=True)
                    nc.tensor.matmul(skps[64:128, :], wpxa_sb[:],
                                     ptaug1[0:65, pr * 256 + 128:(pr + 1) * 256],
                                     start=True, stop=True)
                    res = rot.tile([P, P], F32, tag="res")
                    nc.vector.tensor_add(res[:], ypair[:], skps[:])
                    rT_ps = psB.tile([P, P], F32, tag="rT")
                    nc.tensor.transpose(rT_ps[:], res[:], ident[:])
                    osb = rot.tile([P, P], F32, tag="osb")
                    nc.scalar.activation(osb[:], rT_ps[:], AF.Copy)
                    nc.sync.dma_start(pos_next[pr * 256:pr * 256 + 128, :],
                                      osb[:, 0:64])
                    nc.sync.dma_start(pos_next[pr * 256 + 128:(pr + 1) * 256, :],
                                      osb[:, 64:128])
    nc.compile()
    return nc


_NC_CACHE = None


def _get_nc():
    global _NC_CACHE
    if _NC_CACHE is None:
        _NC_CACHE = build()
    return _NC_CACHE


def _run(inputs, trace=False):
    pos_feat = np.ascontiguousarray(inputs["pos_feat"], dtype=np.float32)
    ene_feat = np.ascontiguousarray(inputs["ene_feat"], dtype=np.float32)
    wx1 = np.ascontiguousarray(inputs["Wx1"], dtype=np.float32)
    wx2 = np.ascontiguousarray(inputs["Wx2"], dtype=np.float32)
    wpx_aug = np.concatenate(
        [np.asarray(inputs["Wpx"], np.float32),
         (np.asarray(inputs["bpx"], np.float32)
          + np.asarray(inputs["bx2"], np.float32))[None, :]], axis=0)
    wte = np.ascontiguousarray(inputs["Wte"], dtype=np.float32)
    wpe_aug = np.concatenate(
        [np.asarray(inputs["Wpe"], np.float32)
         - np.asarray(inputs["Wte"], np.float32),
         (np.asarray(inputs["bpe"], np.float32)
          + np.asarray(inputs["bte"], np.float32))[None, :]], axis=0)
    gx = np.ascontiguousarray(np.asarray(inputs["gx"], np.float32).reshape(PIN, 1))
    betax = np.ascontiguousarray(
        np.asarray(inputs["betax"], np.float32).reshape(PIN, 1))
    assert int(inputs.get("k", K)) == K

    nc = _get_nc()
    in_maps = []
    for c in range(N_CORES):
        in_maps.append({
            "pos": np.ascontiguousarray(pos_feat[c]),
            "ene": np.ascontiguousarray(ene_feat[c]),
            "wx1": wx1, "wx2": wx2, "wpx_aug": np.ascontiguousarray(wpx_aug),
            "wte": wte, "wpe_aug": np.ascontiguousarray(wpe_aug),
            "gx": gx, "betax": betax,
        })
    res = run_bass_kernel_spmd(nc, in_maps, core_ids=list(range(N_CORES)),
                               trace=trace)
    pos_out = np.stack([res.results[c]["pos_next"] for c in range(N_CORES)])
    ene_out = np.stack([res.results[c]["ene_next"] for c in range(N_CORES)])
    return (pos_out, ene_out), res


def kernel(**inputs):
    out, _ = _run(inputs, trace=False)
    return out


# revision 24
# speedup vs baseline: 2.6044x; 1.0033x over previous
# DualEdgeConv Trainium2 Bass kernel.
# Data-parallel over batch: 1 batch per NeuronCore (B=8, 8 cores).
# Per core: KNN (augmented matmul + MAX8/FIND_INDEX8/MATCH_REPLACE8 top-16),
# on-chip ap_gather edge gathers, global BN stats via AllReduce collective.
import sys

sys.path.insert(0, "/opt/trn_rl_repo")

import numpy as np

import concourse.bass as bass
import concourse.mybir as mybir
import concourse.tile as tile
from concourse import bacc
from concourse.bass_utils import run_bass_kernel_spmd
from concourse.masks import make_identity

N_CORES = 8
N = 4096          # nodes per batch
PIN = 64          # pos feature dim
EIN = 32          # ene feature dim
K = 16            # neighbors
P = 128           # partitions
NT = N // P       # 32 node-tiles
NPAIR = N // 256  # 16 edge pairs (256 nodes / 4096 edges each)
NQUAD = N // 512  # 8 quads
NEDGE_TOT = N_CORES * N * K  # 524288 (global BN count)
EPS = 1e-5
F32 = mybir.dt.float32
I16 = mybir.dt.int16
U32 = mybir.dt.uint32
ALU = mybir.AluOpType
AF = mybir.ActivationFunctionType
AX = mybir.AxisListType
NEG_BIG = -3.0e38


def _repl_const(kind):
    """Replication matrices for wrapped-index construction.

    idxT rows q = 16*j + k hold (row-tile j-in-quad, neighbor k).
    u-pair X (X in 0,1): out[p, e] uses idxT row 16*(2X + p//64) + p%16.
    e-quad: out[p, e] uses idxT row 16*(p//32) + p%16.
    """
    cols = 128
    r = np.zeros((64, cols), np.float32)
    for q in range(64):
        j, kk = q // 16, q % 16
        for p in range(cols):
            if p % 16 != kk:
                continue
            if kind == "uA" and j == p // 64:
                r[q, p] = 1.0
            elif kind == "uB" and j == 2 + p // 64:
                r[q, p] = 1.0
            elif kind == "e4" and j == p // 32:
                r[q, p] = 1.0
    return r


def build():
    nc = bacc.Bacc("TRN2", target_bir_lowering=False, debug=False,
                   enable_asserts=False, num_devices=N_CORES)

    pos = nc.dram_tensor("pos", [N, PIN], F32, kind="ExternalInput")
    ene = nc.dram_tensor("ene", [N, EIN], F32, kind="ExternalInput")
    wx1 = nc.dram_tensor("wx1", [PIN, PIN], F32, kind="ExternalInput")
    wx2 = nc.dram_tensor("wx2", [PIN, PIN], F32, kind="ExternalInput")
    wpx_aug = nc.dram_tensor("wpx_aug", [PIN + 1, PIN], F32, kind="ExternalInput")
    wte = nc.dram_tensor("wte", [EIN, EIN], F32, kind="ExternalInput")
    wpe_aug = nc.dram_tensor("wpe_aug", [EIN + 1, EIN], F32, kind="ExternalInput")
    gx = nc.dram_tensor("gx", [PIN, 1], F32, kind="ExternalInput")
    betax = nc.dram_tensor("betax", [PIN, 1], F32, kind="ExternalInput")
    pos_next = nc.dram_tensor("pos_next", [N, PIN], F32, kind="ExternalOutput")
    ene_next = nc.dram_tensor("ene_next", [N, EIN], F32, kind="ExternalOutput")

    ru_a = nc.inline_tensor(_repl_const("uA"), name="ru_a")
    ru_b = nc.inline_tensor(_repl_const("uB"), name="ru_b")
    rq_e = nc.inline_tensor(_repl_const("e4"), name="rq_e")

    uj_spill = [nc.dram_tensor(f"ujsp_{pr}", [P, 2048], F32, kind="Internal")
                for pr in range(NPAIR)]

    with tile.TileContext(nc) as tc:
        with tc.tile_pool(name="singles", bufs=1) as sg, \
             tc.tile_pool(name="rot", bufs=2) as rot:
            # ---------- constants ----------
            ident = sg.tile([P, P], F32)
            make_identity(nc, ident[:])
            eyeneg = sg.tile([P, P], F32)
            nc.gpsimd.memset(eyeneg[:], 0.0)
            nc.gpsimd.affine_select(
                out=eyeneg[:], in_=eyeneg[:], compare_op=ALU.not_equal,
                fill=-1.0e9, base=0, pattern=[[-1, P]], channel_multiplier=1)
            rua_sb = sg.tile([64, P], F32)
            nc.sync.dma_start(rua_sb[:], ru_a.ap())
            rub_sb = sg.tile([64, P], F32)
            nc.sync.dma_start(rub_sb[:], ru_b.ap())
            rqe_sb = sg.tile([64, P], F32)
            nc.sync.dma_start(rqe_sb[:], rq_e.ap())

            # ---------- weights ----------
            wx1_sb = sg.tile([PIN, PIN], F32)
            nc.sync.dma_start(wx1_sb[:], wx1[:])
            bd2 = sg.tile([P, P], F32)
            nc.vector.memset(bd2[:], 0.0)
            nc.sync.dma_start(bd2[0:64, 0:64], wx2[:])
            nc.sync.dma_start(bd2[64:128, 64:128], wx2[:])
            wpxa_sb = sg.tile([PIN + 1, PIN], F32)
            nc.sync.dma_start(wpxa_sb[:], wpx_aug[:])
            wte_sb = sg.tile([EIN, EIN], F32)
            nc.sync.dma_start(wte_sb[:], wte[:])
            wpea_sb = sg.tile([EIN + 1, EIN], F32)
            nc.sync.dma_start(wpea_sb[:], wpe_aug[:])
            gx_sb = sg.tile([PIN, 1], F32)
            nc.sync.dma_start(gx_sb[:], gx[:])
            betax_sb = sg.tile([PIN, 1], F32)
            nc.sync.dma_start(betax_sb[:], betax[:])

            # ---------- persistent big tables ----------
            ptaug1 = sg.tile([96, N], F32)     # rows 0..63 posT, 64.. ones
            ptaug2 = sg.tile([65, N], F32)     # rows 0..63 2*posT, 64 = -sq
            tu = sg.tile([P, N], F32)          # U^T duplicated 2x
            tew = sg.tile([P, N], F32)         # (e@Wte)^T duplicated 4x
            etaug = sg.tile([64, N], F32)      # rows 0..31 e^T, 32.. ones
            nc.vector.memset(ptaug1[64:96, :], 1.0)
            nc.vector.memset(etaug[32:64, :], 1.0)

            uw_all = sg.tile([P, NPAIR * P], I16)   # wrapped neighbor lists (u)
            ew_all = sg.tile([P, NQUAD * P], I16)   # wrapped neighbor lists (e)
            stash = sg.tile([P, NT * K], F32)       # idx per row-tile as f32
            statsz = sg.tile([P, NPAIR], F32)
            statsz2 = sg.tile([P, NPAIR], F32)

            with tc.tile_pool(name="psA", bufs=1, space="PSUM") as psA:
                # ---------- phase A: transposes + tables ----------
                for t in range(NT):
                    prt = rot.tile([P, PIN], F32, tag="prt")
                    nc.sync.dma_start(prt[:], pos[t * P:(t + 1) * P, :])
                    pt_ps = psA.tile([PIN, P], F32, tag="small", bufs=2)
                    nc.tensor.transpose(pt_ps[:], prt[:], ident[:])
                    nc.scalar.activation(ptaug1[0:64, t * P:(t + 1) * P], pt_ps[:],
                                         AF.Copy)
                    nc.scalar.activation(ptaug2[0:64, t * P:(t + 1) * P], pt_ps[:],
                                         AF.Copy, scale=2.0)
                    sq_col = rot.tile([P, 1], F32, tag="sqcol")
                    sq_scr = rot.tile([P, PIN], F32, tag="sqscr")
                    nc.scalar.activation(sq_scr[:], prt[:], AF.Square,
                                         accum_out=sq_col[:])
                    sqT_ps = psA.tile([1, P], F32, tag="small", bufs=2)
                    nc.tensor.transpose(sqT_ps[:], sq_col[:], ident[:])
                    nc.scalar.activation(ptaug2[64:65, t * P:(t + 1) * P], sqT_ps[:],
                                         AF.Copy, scale=-1.0)
                    ert = rot.tile([P, EIN], F32, tag="ert")
                    nc.sync.dma_start(ert[:], ene[t * P:(t + 1) * P, :])
                    et_ps = psA.tile([32, P], F32, tag="small", bufs=2)
                    nc.tensor.transpose(et_ps[:], ert[:], ident[:])
                    nc.scalar.activation(etaug[0:32, t * P:(t + 1) * P], et_ps[:],
                                         AF.Copy)

                # U^T = Wx1^T posT, duplicated into both halves of tu
                for cb in range(N // 512):
                    ups = psA.tile([P, 512], F32, tag="vps", bufs=4)
                    nc.tensor.matmul(ups[0:64, :], wx1_sb[:],
                                     ptaug1[0:64, cb * 512:(cb + 1) * 512],
                                     start=True, stop=True)
                    nc.tensor.matmul(ups[64:128, :], wx1_sb[:],
                                     ptaug1[0:64, cb * 512:(cb + 1) * 512],
                                     start=True, stop=True)
                    nc.scalar.activation(tu[:, cb * 512:(cb + 1) * 512], ups[:],
                                         AF.Copy)
                    eups = psA.tile([32, 512], F32, tag="small", bufs=2,
                                    name=f"eups_{cb}")
                    nc.tensor.matmul(eups[:], wte_sb[:],
                                     etaug[0:32, cb * 512:(cb + 1) * 512],
                                     start=True, stop=True)
                    nc.scalar.activation(tew[0:32, cb * 512:(cb + 1) * 512],
                                         eups[:], AF.Copy)
                for gg in range(1, 4):
                    nc.sync.dma_start(tew[32 * gg:32 * gg + 32, :], tew[0:32, :])

                # ---------- KNN with software-pipelined wraps/gathers/stats ----------
                def emit_uis(pr, nm):
                    # ui strip: [128,128] = U columns for this pair's nodes,
                    # chunk A nodes on partitions 0:64, chunk B on 64:128
                    uis = rot.tile([P, P], F32, tag="uis", bufs=4,
                                   name=f"uis_{nm}_{pr}")
                    nc.sync.dma_start(uis[0:64, :],
                                      tu[0:64, pr * 256:pr * 256 + 128])
                    nc.sync.dma_start(uis[64:128, :],
                                      tu[0:64, pr * 256 + 128:(pr + 1) * 256])
                    return uis

                def emit_ugather(qq):
                    for h in range(2):
                        pr = 2 * qq + h
                        ujt = rot.tile([P, 2048], F32, tag="ujt", bufs=2,
                                       name=f"ujt_s1_{pr}")
                        nc.gpsimd.ap_gather(
                            ujt[:], tu[:], uw_all[:, pr * P:(pr + 1) * P],
                            channels=P, num_elems=N, d=1, num_idxs=2048)
                        nc.sync.dma_start(uj_spill[pr].ap(), ujt[:])

                def emit_ureload(qq):
                    tiles = []
                    for h in range(2):
                        pr = 2 * qq + h
                        ujr = rot.tile([P, 2048], F32, tag="ujt", bufs=2,
                                       name=f"ujr_{pr}")
                        nc.sync.dma_start(ujr[:], uj_spill[pr].ap())
                        tiles.append(ujr)
                    return tiles

                ujr_tiles = {}

                def emit_ustats(qq):
                    for h in range(2):
                        pr = 2 * qq + h
                        ujr = ujr_tiles[qq][h]
                        uis = emit_uis(pr, "s1")
                        zt = rot.tile([P, 2048], F32, tag="row2", bufs=2,
                                      name=f"zt_s1_{pr}")
                        nc.vector.scalar_tensor_tensor(
                            zt[:].rearrange("p (n k) -> p n k", k=K),
                            ujr[:].rearrange("p (n k) -> p n k", k=K), 1.0,
                            uis[:].to_broadcast([P, P, K]),
                            op0=ALU.mult, op1=ALU.subtract,
                            accum_out=statsz[:, pr:pr + 1])
                        nc.scalar.activation(ujr[:], zt[:], AF.Square,
                                             accum_out=statsz2[:, pr:pr + 1])
                    del ujr_tiles[qq]

                def emit_wraps(qq):
                    ixT_ps = psA.tile([64, P], F32, tag="small", bufs=2,
                                      name=f"ixtps_{qq}")
                    nc.tensor.transpose(ixT_ps[:],
                                        stash[:, 4 * qq * K:(4 * qq + 4) * K],
                                        ident[:])
                    ixT = rot.tile([64, P], F32, tag="ixT", bufs=1,
                                   name=f"ixt_{qq}")
                    nc.scalar.activation(ixT[:], ixT_ps[:], AF.Copy)
                    for nm, cst, dst in (
                            ("wa", rua_sb, uw_all[:, (2 * qq) * P:(2 * qq + 1) * P]),
                            ("wb", rub_sb,
                             uw_all[:, (2 * qq + 1) * P:(2 * qq + 2) * P])):
                        wps = psA.tile([P, P], F32, tag="small", bufs=2,
                                       name=f"{nm}_{qq}")
                        nc.tensor.matmul(wps[:], cst[:], ixT[:], start=True,
                                         stop=True)
                        nc.vector.tensor_copy(dst, wps[:])
                    weps = psA.tile([P, P], F32, tag="small", bufs=2,
                                    name=f"we_{qq}")
                    nc.tensor.matmul(weps[:], rqe_sb[:], ixT[:], start=True,
                                     stop=True)
                    nc.vector.tensor_copy(ew_all[:, qq * P:(qq + 1) * P], weps[:])

                def emit_egather(qq):
                    ejt = rot.tile([P, 2048], F32, tag="ejt", bufs=2,
                                   name=f"ejt_{qq}")
                    nc.gpsimd.ap_gather(
                        ejt[:], tew[:], ew_all[:, qq * P:(qq + 1) * P],
                        channels=P, num_elems=N, d=1, num_idxs=2048)
                    return ejt

                ejt_tiles = {}

                def emit_eproc(qq):
                    ejt = ejt_tiles.pop(qq)
                    esum = rot.tile([P, P], F32, tag="esum", bufs=2,
                                    name=f"esum_{qq}")
                    nc.vector.reduce_sum(
                        esum[:], ejt[:].rearrange("p (a b) -> p a b", b=K),
                        axis=AX.X)
                    ev = rot.tile([P, P], F32, tag="ev", bufs=2,
                                  name=f"ev_{qq}")
                    nc.vector.tensor_scalar_mul(ev[:], esum[:], 1.0 / K)
                    # skip slice (e @ (Wpe - Wte) + bte + bpe)^T for this quad
                    skq = psA.tile([32, 512], F32, tag="small", bufs=2,
                                   name=f"skq_{qq}")
                    nc.tensor.matmul(skq[:], wpea_sb[:],
                                     etaug[0:33, qq * 512:(qq + 1) * 512],
                                     start=True, stop=True)
                    skq_sb = rot.tile([32, 512], F32, tag="skqsb", bufs=2,
                                      name=f"skqsb_{qq}")
                    nc.scalar.activation(skq_sb[:], skq[:], AF.Copy)
                    # transpose ev and accumulate the 4 skip transposes
                    rT = psA.tile([P, P], F32, tag="erT", bufs=1,
                                  name=f"erT_{qq}")
                    nc.tensor.transpose(rT[:], ev[:], ident[:])
                    rT2 = psA.tile([P, P], F32, tag="erT2", bufs=1,
                                   name=f"erT2_{qq}")
                    for c in range(4):
                        nc.tensor.transpose(rT2[:, 32 * c:32 * c + 32],
                                            skq_sb[:, c * P:(c + 1) * P],
                                            ident[0:32, 0:32])
                    sk_sb = rot.tile([P, P], F32, tag="sksb", bufs=2,
                                     name=f"sksb_{qq}")
                    nc.scalar.activation(sk_sb[:], rT2[:], AF.Copy)
                    esb2 = rot.tile([P, P], F32, tag="esb2", bufs=2,
                                    name=f"esb2_{qq}")
                    nc.vector.tensor_add(esb2[:], rT[:], sk_sb[:])
                    for c in range(4):
                        nc.sync.dma_start(
                            ene_next[qq * 512 + c * P:qq * 512 + (c + 1) * P, :],
                            esb2[:, 32 * c:32 * c + 32])

                for q in range(NQUAD):
                    if q >= 3:
                        ujr_tiles[q - 3] = emit_ureload(q - 3)
                    for j in range(4):
                        if j == 1 and q >= 1:
                            emit_wraps(q - 1)
                            emit_ugather(q - 1)
                            ejt_tiles[q - 1] = emit_egather(q - 1)
                        r = 4 * q + j
                        row1 = rot.tile([P, N], F32, tag="row1", bufs=2)
                        for cb in range(N // 512):
                            vps = psA.tile([P, 512], F32, tag="vps", bufs=4)
                            nc.tensor.matmul(
                                vps[:], ptaug1[0:65, r * P:(r + 1) * P],
                                ptaug2[0:65, cb * 512:(cb + 1) * 512],
                                start=True, stop=True)
                            nc.scalar.activation(row1[:, cb * 512:(cb + 1) * 512],
                                                 vps[:], AF.Copy)
                        nc.vector.tensor_add(row1[:, r * P:(r + 1) * P],
                                             row1[:, r * P:(r + 1) * P], eyeneg[:])
                        v8a = rot.tile([P, 8], F32, tag="v8a")
                        v8b = rot.tile([P, 8], F32, tag="v8b")
                        i8 = rot.tile([P, K], U32, tag="i8")
                        row2 = rot.tile([P, N], F32, tag="row2", bufs=2)
                        nc.vector.max(out=v8a[:], in_=row1[:])
                        nc.vector.max_index(out=i8[:, 0:8], in_max=v8a[:],
                                            in_values=row1[:])
                        nc.vector.match_replace(out=row2[:], in_to_replace=v8a[:],
                                                in_values=row1[:], imm_value=NEG_BIG)
                        nc.vector.max(out=v8b[:], in_=row2[:])
                        nc.vector.max_index(out=i8[:, 8:16], in_max=v8b[:],
                                            in_values=row2[:])
                        nc.vector.tensor_copy(stash[:, r * K:(r + 1) * K], i8[:])
                    if q >= 3:
                        emit_ustats(q - 3)
                        emit_eproc(q - 3)
                emit_wraps(NQUAD - 1)
                emit_ugather(NQUAD - 1)
                ejt_tiles[NQUAD - 1] = emit_egather(NQUAD - 1)
                for q in range(NQUAD - 3, NQUAD):
                    ujr_tiles[q] = emit_ureload(q)
                    emit_ustats(q)
                    emit_eproc(q)

                # ---------- global BN stats (collective) ----------
                ssum = sg.tile([P, 2], F32)
                nc.vector.reduce_sum(ssum[:, 0:1], statsz[:], axis=AX.X)
                nc.vector.reduce_sum(ssum[:, 1:2], statsz2[:], axis=AX.X)
                comb = sg.tile([64, 2], F32)
                hi = sg.tile([64, 2], F32)
                nc.sync.dma_start(hi[:], ssum[64:128, :])
                nc.vector.tensor_add(comb[:], ssum[0:64, :], hi[:])
            # psA released here

            with tc.tile_pool(name="dramp", bufs=1, space="DRAM") as dramp:
                cc_in = dramp.tile([64, 2], F32)
                cc_out = dramp.tile([64, 2], F32)
                nc.sync.dma_start(cc_in[:], comb[:])
                nc.gpsimd.collective_compute(
                    "AllReduce", ALU.add,
                    replica_groups=[list(range(N_CORES))],
                    ins=[cc_in.opt()], outs=[cc_out.opt()])
                gstat = sg.tile([64, 2], F32)
                nc.sync.dma_start(gstat[:], cc_out[:])

            # BN coefficients a (scale), c (bias), duplicated to both halves
            mmean = sg.tile([64, 1], F32)
            nc.vector.tensor_scalar_mul(mmean[:], gstat[:, 0:1], 1.0 / NEDGE_TOT)
            msq = sg.tile([64, 1], F32)
            nc.vector.tensor_scalar_mul(msq[:], gstat[:, 1:2], 1.0 / NEDGE_TOT)
            nvar = sg.tile([64, 1], F32)   # m^2 - E[z^2]  (negated variance)
            nc.vector.scalar_tensor_tensor(nvar[:], mmean[:], mmean[:], msq[:],
                                           op0=ALU.mult, op1=ALU.subtract)
            epst = sg.tile([64, 1], F32)
            nc.vector.memset(epst[:], EPS)
            sdev = sg.tile([64, 1], F32)
            nc.scalar.activation(sdev[:], nvar[:], AF.Sqrt, bias=epst[:], scale=-1.0)
            inv = sg.tile([64, 1], F32)
            nc.vector.reciprocal(inv[:], sdev[:])
            a_full = sg.tile([P, 1], F32)
            c_full = sg.tile([P, 1], F32)
            nc.vector.tensor_tensor(a_full[0:64, :], inv[:], gx_sb[:], op=ALU.mult)
            ma = sg.tile([64, 1], F32)
            nc.vector.tensor_tensor(ma[:], mmean[:], a_full[0:64, :], op=ALU.mult)
            nc.vector.tensor_tensor(c_full[0:64, :], betax_sb[:], ma[:],
                                    op=ALU.subtract)
            nc.sync.dma_start(a_full[64:128, :], a_full[0:64, :])
            nc.sync.dma_start(c_full[64:128, :], c_full[0:64, :])

            # ---------- stage 2 ----------
            with tc.tile_pool(name="psB", bufs=1, space="PSUM") as psB:
                for pr in range(NPAIR):
                    ujt = rot.tile([P, 2048], F32, tag="ujt", bufs=2)
                    nc.scalar.dma_start(ujt[:], uj_spill[pr].ap())
                    uis = rot.tile([P, P], F32, tag="uis", bufs=4,
                                   name=f"uis_s2_{pr}")
                    nc.scalar.dma_start(uis[0:64, :],
                                        tu[0:64, pr * 256:pr * 256 + 128])
                    nc.scalar.dma_start(uis[64:128, :],
                                        tu[0:64, pr * 256 + 128:(pr + 1) * 256])
                    zt = rot.tile([P, 2048], F32, tag="row2", bufs=2)
                    nc.vector.scalar_tensor_tensor(
                        zt[:].rearrange("p (n k) -> p n k", k=K),
                        ujt[:].rearrange("p (n k) -> p n k", k=K), 1.0,
                        uis[:].to_broadcast([P, P, K]),
                        op0=ALU.mult, op1=ALU.subtract)
                    nc.scalar.activation(zt[:], zt[:], AF.Relu, bias=c_full[:],
                                         scale=a_full[:])
                    ypair = rot.tile([P, P], F32, tag="ypair")
                    for j in range(4):
                        h2ps = psB.tile([P, 512], F32, tag="h2", bufs=2)
                        nc.tensor.matmul(h2ps[:], bd2[:],
                                         zt[:, j * 512:(j + 1) * 512],
                                         start=True, stop=True)
                        nc.vector.reduce_max(
                            ypair[:, j * 32:(j + 1) * 32],
                            h2ps[:].rearrange("p (a b) -> p a b", b=K), axis=AX.X)
                    skps = psB.tile([P, P], F32, tag="skps")
                    nc.tensor.matmul(skps[0:64, :], wpxa_sb[:],
                                     ptaug1[0:65, pr * 256:pr * 256 + 128],
                                     start=True, stop=True)
                    nc.tensor.matmul(skps[64:128, :], wpxa_sb[:],
                                     ptaug1[0:65, pr * 256 + 128:(pr + 1) * 256],
                                     start=True, stop=True)
                    res = rot.tile([P, P], F32, tag="res")
                    nc.vector.tensor_add(res[:], ypair[:], skps[:])
                    rT_ps = psB.tile([P, P], F32, tag="rT")
                    nc.tensor.transpose(rT_ps[:], res[:], ident[:])
                    osb = rot.tile([P, P], F32, tag="osb")
                    nc.scalar.activation(osb[:], rT_ps[:], AF.Copy)
                    nc.sync.dma_start(pos_next[pr * 256:pr * 256 + 128, :],
                                      osb[:, 0:64])
                    nc.sync.dma_start(pos_next[pr * 256 + 128:(pr + 1) * 256, :],
                                      osb[:, 64:128])
    nc.compile()
    return nc


_NC_CACHE = None


def _get_nc():
    global _NC_CACHE
    if _NC_CACHE is None:
        _NC_CACHE = build()
    return _NC_CACHE


def _run(inputs, trace=False):
    pos_feat = np.ascontiguousarray(inputs["pos_feat"], dtype=np.float32)
    ene_feat = np.ascontiguousarray(inputs["ene_feat"], dtype=np.float32)
    wx1 = np.ascontiguousarray(inputs["Wx1"], dtype=np.float32)
    wx2 = np.ascontiguousarray(inputs["Wx2"], dtype=np.float32)
    wpx_aug = np.concatenate(
        [np.asarray(inputs["Wpx"], np.float32),
         (np.asarray(inputs["bpx"], np.float32)
          + np.asarray(inputs["bx2"], np.float32))[None, :]], axis=0)
    wte = np.ascontiguousarray(inputs["Wte"], dtype=np.float32)
    wpe_aug = np.concatenate(
        [np.asarray(inputs["Wpe"], np.float32)
         - np.asarray(inputs["Wte"], np.float32),
         (np.asarray(inputs["bpe"], np.float32)
          + np.asarray(inputs["bte"], np.float32))[None, :]], axis=0)
    gx = np.ascontiguousarray(np.asarray(inputs["gx"], np.float32).reshape(PIN, 1))
    betax = np.ascontiguousarray(
        np.asarray(inputs["betax"], np.float32).reshape(PIN, 1))
    assert int(inputs.get("k", K)) == K

    nc = _get_nc()
    in_maps = []
    for c in range(N_CORES):
        in_maps.append({
            "pos": np.ascontiguousarray(pos_feat[c]),
            "ene": np.ascontiguousarray(ene_feat[c]),
            "wx1": wx1, "wx2": wx2, "wpx_aug": np.ascontiguousarray(wpx_aug),
            "wte": wte, "wpe_aug": np.ascontiguousarray(wpe_aug),
            "gx": gx, "betax": betax,
        })
    res = run_bass_kernel_spmd(nc, in_maps, core_ids=list(range(N_CORES)),
                               trace=trace)
    pos_out = np.stack([res.results[c]["pos_next"] for c in range(N_CORES)])
    ene_out = np.stack([res.results[c]["ene_next"] for c in range(N_CORES)])
    return (pos_out, ene_out), res


def kernel(**inputs):
    out, _ = _run(inputs, trace=False)
    return out
